# revision 5
# baseline (speedup 1.0000x reference)
"""Trainium2 Bass kernel for BBoxHeadForGroundTruthBboxRegressionV2.

Strategy
--------
Data-parallel over the 256 ground tokens: 8 NeuronCores x 32 tokens.
Each core runs, fully on-device, for its 32 token-sequences of length
257 (1 lang token + 256 vision latents, padded to 258 columns):

  lang-projection MLP + per-token vision projection  -> x  [1024, 32*258]
  4 pre-LN transformer encoder layers (H=4, dh=256)
  final LN + 5-layer box MLP on the fused (first) token -> [6, 32]

All matmuls run in float32r (fp32 rounded to 11 mantissa bits, processed
at bf16 rate by the PE; exact products, fp32 accumulation).  Activations
are kept feature-major ([d on partitions, tokens on free]); LayerNorm
statistics are computed with ones-matmuls on the PE and broadcast back
across partitions with rank-1 matmuls.  LN gamma/beta, attention scale
and the V bias are folded into weights on the host.  Layer 3 computes
attention only for the fused query column, and the final FF layer + box
head run on the compacted [1024, 32] fused matrix.
"""

import numpy as np

N_CORES = 8
T = 256
D_LM = 4096
D_VIS = 1024
D = 1024
L = 256
B = 16
FF = 1024
H = 4
NL = 4
EPS = 1e-5

TS = T // N_CORES      # tokens per core
S = 258                # padded sequence length (1 lang + 256 vis + 1 pad)
SR = 257               # real sequence length
F = TS * S             # flattened columns per core
DC = D // 128          # feature chunks (8)

_COMPILED = None


def _round_f32r(x):
    """Round float32 array to 11 mantissa bits (RNE) == float32r grid."""
    x = np.ascontiguousarray(x, dtype=np.float32)
    bits = x.view(np.uint32).astype(np.uint64)
    half = np.uint64(2047)
    lsb = (bits >> np.uint64(12)) & np.uint64(1)
    bits = (bits + half + lsb) >> np.uint64(12) << np.uint64(12)
    return bits.astype(np.uint32).view(np.float32)


def _bias_cols(b):
    """[n*128] bias vector -> [128, n] (column per 128-feature chunk)."""
    b = np.asarray(b, dtype=np.float32)
    return np.ascontiguousarray(b.reshape(-1, 128).T)


def _prep_weights(params):
    p = {k: np.asarray(v, dtype=np.float32) for k, v in params.items()}
    w = {}
    r = _round_f32r

    w["vis_WT"] = r(p["vis_W"].T)                      # [D_VIS, D]
    w["vis_b"] = _bias_cols(p["vis_b"])

    w["lang_W1T"] = r(p["lang_W1"].T)                  # [4096, 1024]
    w["lang_W2T"] = r(p["lang_W2"].T)
    w["lang_W3T"] = r(p["lang_W3"].T)
    w["lang_b1"] = _bias_cols(p["lang_b1"])
    w["lang_b2"] = _bias_cols(p["lang_b2"])
    w["lang_b3"] = _bias_cols(p["lang_b3"])

    for i in range(NL):
        g1 = p["enc_ln1_g"][i]
        be1 = p["enc_ln1_b"][i]
        Wqkv = p["enc_Wqkv"][i]
        bqkv = p["enc_bqkv"][i]
        Wq, Wk, Wv = Wqkv[0:D], Wqkv[D:2 * D], Wqkv[2 * D:3 * D]
        bq, bk, bv = bqkv[0:D], bqkv[D:2 * D], bqkv[2 * D:3 * D]

        Wq_f = Wq * g1[None, :]
        b_q = bq + Wq @ be1
        Wk_f = Wk * g1[None, :] / 16.0
        b_k = (bk + Wk @ be1) / 16.0
        Wv_f = Wv * g1[None, :]
        b_v = bv + Wv @ be1
        Wo = p["enc_Wo"][i]
        b_o = p["enc_bo"][i] + Wo @ b_v

        g2 = p["enc_ln2_g"][i]
        be2 = p["enc_ln2_b"][i]
        Wf1_f = p["enc_Wff1"][i] * g2[None, :]
        b_f1 = p["enc_bff1"][i] + p["enc_Wff1"][i] @ be2

        w[f"l{i}_WqT"] = r(Wq_f.T)
        w[f"l{i}_WkT"] = r(Wk_f.T)
        w[f"l{i}_WvT"] = r(Wv_f.T)
        w[f"l{i}_WoT"] = r(Wo.T)
        w[f"l{i}_bq"] = _bias_cols(b_q)
        w[f"l{i}_bk"] = _bias_cols(b_k)
        w[f"l{i}_bo"] = _bias_cols(b_o)
        w[f"l{i}_Wf1T"] = r(Wf1_f.T)
        w[f"l{i}_Wf2T"] = r(p["enc_Wff2"][i].T)
        w[f"l{i}_bf1"] = _bias_cols(b_f1)
        w[f"l{i}_bf2"] = _bias_cols(p["enc_bff2"][i])

    gf = p["enc_lnf_g"]
    bef = p["enc_lnf_b"]
    W1_f = p["box_W1"] * gf[None, :]
    b_1 = p["box_b1"] + p["box_W1"] @ bef
    w["box_W1T"] = r(W1_f.T)
    w["box_b1"] = _bias_cols(b_1)
    for j in (2, 3, 4):
        w[f"box_W{j}T"] = r(p[f"box_W{j}"].T)
        w[f"box_b{j}"] = _bias_cols(p[f"box_b{j}"])
    w["box_W5T"] = r(p["box_W5"].T)                    # [1024, 6]
    w["box_b5"] = np.ascontiguousarray(p["box_b5"].reshape(6, 1))

    w["consts"] = np.stack(
        [np.ones(128, np.float32), np.full(128, 1.0 / 1024.0, np.float32)], axis=1
    )
    w["ones_row"] = np.ones((1, 128), np.float32)
    w["zeros32"] = np.zeros((128, TS), np.float32)
    w["epsb"] = np.full((128, 1), EPS, np.float32)
    return w


# ---------------------------------------------------------------------------
# graph builder
# ---------------------------------------------------------------------------

def _build():
    import concourse.bacc as bacc
    import concourse.mybir as mybir
    from concourse.tile import TileContext
    import concourse.bass as bass

    f32 = mybir.dt.float32
    f32r = mybir.dt.float32r
    Alu = mybir.AluOpType
    Act = mybir.ActivationFunctionType

    nc = bacc.Bacc("TRN2", target_bir_lowering=False, debug=False,
                   num_devices=N_CORES)

    def din(name, shape, dtype=f32r):
        return nc.dram_tensor(name, list(shape), dtype, kind="ExternalInput")

    grdT = din("grdT", [D_LM, TS])
    visT = din("visT", [D_VIS, TS * L])
    consts = din("consts", [128, 2])
    ones_row_d = din("ones_row", [1, 128])
    zeros_d = din("zeros32", [128, TS])
    epsb_d = din("epsb", [128, 1], f32)
    vis_WT = din("vis_WT", [D_VIS, D])
    vis_b = din("vis_b", [128, DC], f32)
    lang_W1T = din("lang_W1T", [D_LM, FF])
    lang_W2T = din("lang_W2T", [FF, FF])
    lang_W3T = din("lang_W3T", [FF, D])
    lang_b = [din(f"lang_b{j}", [128, DC], f32) for j in (1, 2, 3)]
    lw = {}
    for i in range(NL):
        for nm in ("WqT", "WkT", "WvT", "WoT", "Wf1T", "Wf2T"):
            lw[(i, nm)] = din(f"l{i}_{nm}", [D, D])
        for nm in ("bq", "bk", "bo", "bf1", "bf2"):
            lw[(i, nm)] = din(f"l{i}_{nm}", [128, DC], f32)
    box_WT = {j: din(f"box_W{j}T", [D, D]) for j in (1, 2, 3, 4)}
    box_b = {j: din(f"box_b{j}", [128, DC], f32) for j in (1, 2, 3, 4)}
    box_W5T = din("box_W5T", [D, 6])
    box_b5 = din("box_b5", [6, 1], f32)

    out = nc.dram_tensor("out", [6, TS], f32, kind="ExternalOutput")

    ds = bass.ds

    with TileContext(nc) as tc:
        dram_cm = tc.tile_pool(name="dram", bufs=1, space="DRAM")
        dram = dram_cm.__enter__()
        xA = dram.tile([128, DC * F], f32r, name="xA")
        xB = dram.tile([128, DC * F], f32r, name="xB")
        xFz = dram.tile([128, DC * TS], f32r, name="xFz")   # fused cols after L3A

        cpool_cm = tc.tile_pool(name="cpool", bufs=1)
        cpool = cpool_cm.__enter__()
        ones2 = cpool.tile([128, 2], f32r)
        nc.sync.dma_start(out=ones2[:], in_=consts[:])
        ones_r = cpool.tile([1, 128], f32r)
        nc.sync.dma_start(out=ones_r[:], in_=ones_row_d[:])
        epsb = cpool.tile([128, 1], f32)
        nc.sync.dma_start(out=epsb[:], in_=epsb_d[:])

        # ---------------- helpers ----------------------------------------
        def layernorm(pool, pp, xs, n, tag):
            """feature-major LN over DC chunks of xs ([128, DC*n]); returns
            standardized xh [128, DC*n] f32r (gamma/beta folded into the
            following matmul's weights on the host)."""
            ps_mu = pp.tile([1, n], f32, tag=f"{tag}_mu", name=f"{tag}_mu")
            ps_s2 = pp.tile([1, n], f32, tag=f"{tag}_s2", name=f"{tag}_s2")
            for c in range(DC):
                nc.tensor.matmul(ps_mu[:], ones2[:, 1:2], xs[:, c * n:(c + 1) * n],
                                 start=(c == 0), stop=(c == DC - 1))
            for c in range(DC):
                xsq = pool.tile([128, n], f32r, tag=f"{tag}_xsq",
                                name=f"{tag}_xsq{c}", bufs=2)
                nc.scalar.square(xsq[:], xs[:, c * n:(c + 1) * n])
                nc.tensor.matmul(ps_s2[:], ones2[:, 1:2], xsq[:],
                                 start=(c == 0), stop=(c == DC - 1))
            mu = pool.tile([1, n], f32r, tag=f"{tag}_musb", name=f"{tag}_musb")
            nc.scalar.copy(mu[:], ps_mu[:])
            musq = pool.tile([1, n], f32, tag=f"{tag}_musq", name=f"{tag}_musq")
            nc.scalar.square(musq[:], mu[:])
            var = pool.tile([1, n], f32, tag=f"{tag}_var", name=f"{tag}_var")
            nc.vector.tensor_tensor(out=var[:], in0=ps_s2[:], in1=musq[:],
                                    op=Alu.subtract)
            sd = pool.tile([1, n], f32, tag=f"{tag}_sd", name=f"{tag}_sd")
            nc.scalar.activation(sd[:], var[:], Act.Sqrt, bias=epsb[0:1, :])
            rstd = pool.tile([1, n], f32r, tag=f"{tag}_rstd", name=f"{tag}_rstd")
            with nc.allow_low_precision(reason="f32r rstd for broadcast matmul"):
                nc.vector.reciprocal(rstd[:], sd[:])
            ps_mub = pp.tile([128, n], f32, tag=f"{tag}_mub", name=f"{tag}_mub")
            nc.tensor.matmul(ps_mub[:], ones_r[:], mu[:], start=True, stop=True)
            ps_rb = pp.tile([128, n], f32, tag=f"{tag}_rb", name=f"{tag}_rb")
            nc.tensor.matmul(ps_rb[:], ones_r[:], rstd[:], start=True, stop=True)
            xh = pool.tile([128, DC * n], f32r, tag=f"{tag}_xh", name=f"{tag}_xh")
            for c in range(DC):
                td = pool.tile([128, n], f32, tag=f"{tag}_td",
                               name=f"{tag}_td{c}", bufs=2)
                nc.vector.tensor_tensor(out=td[:], in0=xs[:, c * n:(c + 1) * n],
                                        in1=ps_mub[:], op=Alu.subtract)
                nc.vector.tensor_tensor(out=xh[:, c * n:(c + 1) * n], in0=td[:],
                                        in1=ps_rb[:], op=Alu.mult)
            return xh

        def mm_block(pp, pool, wtiles, rhs_tile, n, bias, act, out_tile, tag,
                     n_dc=DC, n_cc=DC):
            """out[:, cc*n:(cc+1)*n] =
            act(sum_dc wtiles[dc][:, cc*128:+128].T @ rhs[:, dc*n:+n] + bias[cc])"""
            for cc in range(n_cc):
                ps = pp.tile([128, n], f32, tag="acc", name=f"{tag}_ps{cc}", bufs=2)
                for dc in range(n_dc):
                    nc.tensor.matmul(ps[:], wtiles[dc][:, cc * 128:(cc + 1) * 128],
                                     rhs_tile[:, dc * n:(dc + 1) * n],
                                     start=(dc == 0), stop=(dc == n_dc - 1))
                dst = out_tile[:, cc * n:(cc + 1) * n]
                if act == "relu":
                    nc.scalar.activation(dst, ps[:], Act.Relu, bias=bias[:, cc:cc + 1])
                elif act == "bias":
                    nc.vector.tensor_scalar_add(out=dst, in0=ps[:],
                                                scalar1=bias[:, cc:cc + 1])
                else:
                    nc.vector.tensor_copy(out=dst, in_=ps[:])

        def load_w(pool, dram_t, name, ncols=D):
            tiles = []
            for dcv in range(DC):
                t = pool.tile([128, ncols], f32r, tag=f"{name}{dcv}",
                              name=f"{name}{dcv}")
                nc.sync.dma_start(out=t[:], in_=dram_t[dcv * 128:(dcv + 1) * 128, :])
                tiles.append(t)
            return tiles

        def load_b(pool, dram_t, name):
            t = pool.tile([128, DC], f32, name=name)
            nc.sync.dma_start(out=t[:], in_=dram_t[:])
            return t

        # ---------------- stage 0: lang MLP ------------------------------
        with tc.tile_pool(name="lang", bufs=1) as pool, \
             tc.tile_pool(name="lang_ps", bufs=1, space="PSUM") as pp:
            g_sb = pool.tile([128, 32 * TS], f32r)
            nc.sync.dma_start(
                out=g_sb[:].rearrange("p (c f) -> p c f", c=32),
                in_=grdT[:].rearrange("(c p) f -> p c f", p=128))
            w1 = []
            for dcv in range(32):
                t = pool.tile([128, FF], f32r, tag=f"lw1_{dcv}", name=f"lw1_{dcv}")
                nc.sync.dma_start(out=t[:], in_=lang_W1T[dcv * 128:(dcv + 1) * 128, :])
                w1.append(t)
            b1 = load_b(pool, lang_b[0], "lb1")
            b2 = load_b(pool, lang_b[1], "lb2")
            b3 = load_b(pool, lang_b[2], "lb3")
            h1 = pool.tile([128, DC * TS], f32r)
            mm_block(pp, pool, w1, g_sb, TS, b1, "relu", h1, "lg1", n_dc=32)
            w2 = load_w(pool, lang_W2T, "lw2", FF)
            h2 = pool.tile([128, DC * TS], f32r)
            mm_block(pp, pool, w2, h1, TS, b2, "relu", h2, "lg2")
            w3 = load_w(pool, lang_W3T, "lw3", D)
            lang_o = pool.tile([128, DC * TS], f32r)
            mm_block(pp, pool, w3, h2, TS, b3, "bias", lang_o, "lg3")
            zt = pool.tile([128, TS], f32r)
            nc.sync.dma_start(out=zt[:], in_=zeros_d[:])
            for c in range(DC):
                xa_c = xA[:, c * F:(c + 1) * F].rearrange("p (t s) -> p t s", s=S)
                nc.sync.dma_start(out=xa_c[:, :, 0:1],
                                  in_=lang_o[:, c * TS:(c + 1) * TS])
                nc.sync.dma_start(out=xa_c[:, :, SR:S], in_=zt[:])

        # ---------------- stage 1: vision projection ---------------------
        with tc.tile_pool(name="vis", bufs=1) as pool, \
             tc.tile_pool(name="vis_ps", bufs=1, space="PSUM") as pp:
            wv = load_w(pool, vis_WT, "visw", D)
            vb = load_b(pool, vis_b, "visb")
            with tc.For_i(0, TS) as iv:
                v_in = pool.tile([128, DC * L], f32r, tag="v_in", bufs=2)
                for c in range(DC):
                    nc.sync.dma_start(out=v_in[:, c * L:(c + 1) * L],
                                      in_=visT[c * 128:(c + 1) * 128, ds(iv * L, L)])
                v_out = pool.tile([128, DC * L], f32r, tag="v_out", bufs=2)
                mm_block(pp, pool, wv, v_in, L, vb, "bias", v_out, "vis")
                for c in range(DC):
                    nc.sync.dma_start(out=xA[:, ds(c * F + iv * S + 1, L)],
                                      in_=v_out[:, c * L:(c + 1) * L])

        # ---------------- encoder layers ---------------------------------
        def attn_pass(li, xin, xout, trim):
            NQ = 2 if trim else S
            with tc.tile_pool(name=f"l{li}a", bufs=1) as pool, \
                 tc.tile_pool(name=f"l{li}a_ps", bufs=1, space="PSUM") as pp:
                wq = load_w(pool, lw[(li, "WqT")], f"wq{li}")
                wk = load_w(pool, lw[(li, "WkT")], f"wk{li}")
                wv_ = load_w(pool, lw[(li, "WvT")], f"wv{li}")
                wo = load_w(pool, lw[(li, "WoT")], f"wo{li}")
                bq_s = load_b(pool, lw[(li, "bq")], f"bq{li}")
                bk_s = load_b(pool, lw[(li, "bk")], f"bk{li}")
                bo_s = load_b(pool, lw[(li, "bo")], f"bo{li}")

                with tc.For_i(0, TS) as iv:
                    xs = pool.tile([128, DC * S], f32r, tag="xs")
                    for c in range(DC):
                        nc.sync.dma_start(out=xs[:, c * S:(c + 1) * S],
                                          in_=xin[:, ds(c * F + iv * S, S)])
                    xh = layernorm(pool, pp, xs, S, f"ln{li}a")

                    k_sb = pool.tile([128, DC * S], f32r, tag="k_sb")
                    mm_block(pp, pool, wk, xh, S, bk_s, "bias", k_sb, "k")
                    q_sb = pool.tile([128, DC * NQ], f32r, tag="q_sb")
                    if trim:
                        xhq = pool.tile([128, DC * NQ], f32r, tag="xhq")
                        for c in range(DC):
                            nc.vector.tensor_copy(out=xhq[:, c * NQ:(c + 1) * NQ],
                                                  in_=xh[:, c * S:c * S + NQ])
                        mm_block(pp, pool, wq, xhq, NQ, bq_s, "bias", q_sb, "q")
                    else:
                        mm_block(pp, pool, wq, xh, S, bq_s, "bias", q_sb, "q")

                    # V token-major: rows = sequence positions, cols = 4*256
                    v_sb = [pool.tile([128, D], f32r, tag=f"v_sb{fc}",
                                      name=f"v_sb{fc}") for fc in range(3)]
                    for fc, (f0, fn) in enumerate(((0, 128), (128, 128), (256, 2))):
                        for hp in range(2):
                            ps = pp.tile([128, 512], f32, tag="acc",
                                         name=f"vps{fc}_{hp}", bufs=2)
                            for c in range(DC):
                                nc.tensor.matmul(
                                    ps[0:fn, :],
                                    xh[:, c * S + f0:c * S + f0 + fn],
                                    wv_[c][:, hp * 512:(hp + 1) * 512],
                                    start=(c == 0), stop=(c == DC - 1))
                            nc.vector.tensor_copy(
                                out=v_sb[fc][0:fn, hp * 512:(hp + 1) * 512],
                                in_=ps[0:fn, :])

                    on_sb = pool.tile([128, DC * NQ], f32r, tag="on_sb")
                    kchunks = ((0, 128), (128, 128), (256, 1))
                    for h in range(H):
                        es = pool.tile([128, 2 * NQ], f32r, tag="es",
                                       name=f"es{h}", bufs=2)
                        es2 = pool.tile([1, NQ], f32r, tag="es2",
                                        name=f"es2{h}", bufs=2)
                        for kc, (k0, kn) in enumerate(kchunks):
                            ps_st = pp.tile([128, NQ], f32, tag="acc",
                                            name=f"st{h}_{kc}", bufs=2)
                            for d2 in range(2):
                                kt = 2 * h + d2
                                nc.tensor.matmul(
                                    ps_st[0:kn, :],
                                    k_sb[:, kt * S + k0:kt * S + k0 + kn],
                                    q_sb[:, kt * NQ:(kt + 1) * NQ],
                                    start=(d2 == 0), stop=(d2 == 1))
                            tgt = es[0:kn, kc * NQ:(kc + 1) * NQ] if kc < 2 \
                                else es2[0:1, :]
                            nc.scalar.activation(tgt, ps_st[0:kn, :], Act.Exp)
                        ps_den = pp.tile([1, NQ], f32, tag="den", name=f"den{h}")
                        nc.tensor.matmul(ps_den[:], ones2[:, 0:1],
                                         es[:, 0:NQ], start=True, stop=False)
                        nc.tensor.matmul(ps_den[:], ones2[:, 0:1],
                                         es[:, NQ:2 * NQ], start=False, stop=False)
                        nc.tensor.matmul(ps_den[:], ones2[0:1, 0:1],
                                         es2[0:1, :], start=False, stop=True)
                        recip = pool.tile([1, NQ], f32r, tag="recip",
                                          name=f"recip{h}")
                        with nc.allow_low_precision(reason="softmax denom recip"):
                            nc.vector.reciprocal(recip[:], ps_den[:])
                        ps_rb = pp.tile([128, NQ], f32, tag="rbb", name=f"rb{h}")
                        nc.tensor.matmul(ps_rb[:], ones_r[:], recip[:],
                                         start=True, stop=True)
                        rb_sb = pool.tile([128, NQ], f32, tag="rb_sb",
                                          name=f"rb_sb{h}")
                        nc.scalar.copy(rb_sb[:], ps_rb[:])
                        for d2 in range(2):
                            ps_o = pp.tile([128, NQ], f32, tag="acc",
                                           name=f"o{h}_{d2}", bufs=2)
                            for kc, (k0, kn) in enumerate(kchunks):
                                src = es[0:kn, kc * NQ:(kc + 1) * NQ] if kc < 2 \
                                    else es2[0:1, :]
                                nc.tensor.matmul(
                                    ps_o[:],
                                    v_sb[kc][0:kn, h * 256 + d2 * 128:
                                             h * 256 + d2 * 128 + 128],
                                    src, start=(kc == 0), stop=(kc == 2))
                            cc = 2 * h + d2
                            nc.vector.tensor_tensor(
                                out=on_sb[:, cc * NQ:(cc + 1) * NQ],
                                in0=ps_o[:], in1=rb_sb[:], op=Alu.mult)

                    for cc in range(DC):
                        ps = pp.tile([128, NQ], f32, tag="acc",
                                     name=f"wops{cc}", bufs=2)
                        for dcv in range(DC):
                            nc.tensor.matmul(ps[:],
                                             wo[dcv][:, cc * 128:(cc + 1) * 128],
                                             on_sb[:, dcv * NQ:(dcv + 1) * NQ],
                                             start=(dcv == 0), stop=(dcv == DC - 1))
                        xnew = pool.tile([128, NQ], f32r, tag="xnew",
                                         name=f"xnew{cc}", bufs=3)
                        nc.vector.scalar_tensor_tensor(
                            out=xnew[:], in0=ps[:], scalar=bo_s[:, cc:cc + 1],
                            in1=xs[:, cc * S:cc * S + NQ],
                            op0=Alu.add, op1=Alu.add)
                        if trim:
                            nc.sync.dma_start(out=xFz[:, ds(cc * TS + iv, 1)],
                                              in_=xnew[:, 0:1])
                        else:
                            nc.sync.dma_start(out=xout[:, ds(cc * F + iv * S, S)],
                                              in_=xnew[:])

        def ff_pass(li, xin, xout):
            with tc.tile_pool(name=f"l{li}b", bufs=1) as pool, \
                 tc.tile_pool(name=f"l{li}b_ps", bufs=1, space="PSUM") as pp:
                wf1 = load_w(pool, lw[(li, "Wf1T")], f"wf1{li}")
                wf2 = load_w(pool, lw[(li, "Wf2T")], f"wf2{li}")
                bf1_s = load_b(pool, lw[(li, "bf1")], f"bf1{li}")
                bf2_s = load_b(pool, lw[(li, "bf2")], f"bf2{li}")
                with tc.For_i(0, TS) as iv:
                    xs = pool.tile([128, DC * S], f32r, tag="xs")
                    for c in range(DC):
                        nc.sync.dma_start(out=xs[:, c * S:(c + 1) * S],
                                          in_=xin[:, ds(c * F + iv * S, S)])
                    xh = layernorm(pool, pp, xs, S, f"ln{li}b")
                    hmid = pool.tile([128, DC * S], f32r, tag="hmid")
                    mm_block(pp, pool, wf1, xh, S, bf1_s, "relu", hmid, "ff1")
                    for cc in range(DC):
                        ps = pp.tile([128, S], f32, tag="acc",
                                     name=f"f2ps{cc}", bufs=2)
                        for dcv in range(DC):
                            nc.tensor.matmul(ps[:],
                                             wf2[dcv][:, cc * 128:(cc + 1) * 128],
                                             hmid[:, dcv * S:(dcv + 1) * S],
                                             start=(dcv == 0), stop=(dcv == DC - 1))
                        xnew = pool.tile([128, S], f32r, tag="xnew",
                                         name=f"fxnew{cc}", bufs=3)
                        nc.vector.scalar_tensor_tensor(
                            out=xnew[:], in0=ps[:], scalar=bf2_s[:, cc:cc + 1],
                            in1=xs[:, cc * S:(cc + 1) * S],
                            op0=Alu.add, op1=Alu.add)
                        nc.sync.dma_start(out=xout[:, ds(cc * F + iv * S, S)],
                                          in_=xnew[:])

        attn_pass(0, xA, xB, trim=False)
        ff_pass(0, xB, xA)
        attn_pass(1, xA, xB, trim=False)
        ff_pass(1, xB, xA)
        attn_pass(2, xA, xB, trim=False)
        ff_pass(2, xB, xA)
        attn_pass(3, xA, None, trim=True)

        # ---------------- final FF + LN + box head on fused cols ---------
        with tc.tile_pool(name="post", bufs=1) as pool, \
             tc.tile_pool(name="post_ps", bufs=1, space="PSUM") as pp:
            xs = pool.tile([128, DC * TS], f32r)
            nc.sync.dma_start(out=xs[:], in_=xFz[:])
            wf1 = load_w(pool, lw[(3, "Wf1T")], "wf13")
            wf2 = load_w(pool, lw[(3, "Wf2T")], "wf23")
            bf1_s = load_b(pool, lw[(3, "bf1")], "bf13")
            bf2_s = load_b(pool, lw[(3, "bf2")], "bf23")
            xh = layernorm(pool, pp, xs, TS, "lnp")
            hmid = pool.tile([128, DC * TS], f32r)
            mm_block(pp, pool, wf1, xh, TS, bf1_s, "relu", hmid, "pf1")
            xfin = pool.tile([128, DC * TS], f32r)
            for cc in range(DC):
                ps = pp.tile([128, TS], f32, tag="acc", name=f"pf2ps{cc}", bufs=2)
                for dcv in range(DC):
                    nc.tensor.matmul(ps[:], wf2[dcv][:, cc * 128:(cc + 1) * 128],
                                     hmid[:, dcv * TS:(dcv + 1) * TS],
                                     start=(dcv == 0), stop=(dcv == DC - 1))
                nc.vector.scalar_tensor_tensor(
                    out=xfin[:, cc * TS:(cc + 1) * TS], in0=ps[:],
                    scalar=bf2_s[:, cc:cc + 1], in1=xs[:, cc * TS:(cc + 1) * TS],
                    op0=Alu.add, op1=Alu.add)
            xhf = layernorm(pool, pp, xfin, TS, "lnp")
            cur = xhf
            for j in (1, 2, 3, 4):
                wj = load_w(pool, box_WT[j], f"bx{j}")
                bj_s = load_b(pool, box_b[j], f"bxb{j}")
                nxt = pool.tile([128, DC * TS], f32r, name=f"bxh{j}")
                mm_block(pp, pool, wj, cur, TS, bj_s, "relu", nxt, f"bx{j}")
                cur = nxt
            w5 = pool.tile([128, DC * 6], f32r)
            nc.sync.dma_start(
                out=w5[:].rearrange("p (c f) -> p c f", c=DC),
                in_=box_W5T[:].rearrange("(c p) f -> p c f", p=128))
            b5_s = pool.tile([6, 1], f32)
            nc.sync.dma_start(out=b5_s[:], in_=box_b5[:])
            ps5 = pp.tile([6, TS], f32, name="ps5")
            for dcv in range(DC):
                nc.tensor.matmul(ps5[:], w5[:, dcv * 6:(dcv + 1) * 6],
                                 cur[:, dcv * TS:(dcv + 1) * TS],
                                 start=(dcv == 0), stop=(dcv == DC - 1))
            ob = pool.tile([6, TS], f32)
            nc.vector.tensor_scalar_add(out=ob[:], in0=ps5[:], scalar1=b5_s[:])
            nc.sync.dma_start(out=out[:], in_=ob[:])

        cpool_cm.__exit__(None, None, None)
        dram_cm.__exit__(None, None, None)

    nc.compile()
    return nc


def _get_compiled():
    global _COMPILED
    if _COMPILED is None:
        _COMPILED = _build()
    return _COMPILED


# ---------------------------------------------------------------------------
# host entry point
# ---------------------------------------------------------------------------

def kernel(grd_token_hidden_states, batch_idx, vision_features_before_mm_projection,
           params):
    from concourse.bass_utils import run_bass_kernel_spmd

    grd = np.asarray(grd_token_hidden_states, dtype=np.float32)
    bidx = np.asarray(batch_idx).astype(np.int64)
    vis = np.asarray(vision_features_before_mm_projection, dtype=np.float32)

    w = _prep_weights(params)
    nc = _get_compiled()

    in_maps = []
    for i in range(N_CORES):
        sl = slice(i * TS, (i + 1) * TS)
        m = dict(w)
        m["grdT"] = _round_f32r(grd[sl].T)
        gathered = vis[bidx[sl]]                      # [TS, L, D_VIS]
        m["visT"] = _round_f32r(
            gathered.transpose(2, 0, 1).reshape(D_VIS, TS * L))
        in_maps.append(m)

    res = run_bass_kernel_spmd(nc, in_maps, core_ids=list(range(N_CORES)))
    outs = [np.asarray(res.results[i]["out"]).T for i in range(N_CORES)]
    return np.concatenate(outs, axis=0).astype(np.float32)


# revision 7
# speedup vs baseline: 1.0117x; 1.0117x over previous
"""Trainium2 Bass kernel for BBoxHeadForGroundTruthBboxRegressionV2.

Strategy
--------
Data-parallel over the 256 ground tokens: 8 NeuronCores x 32 tokens.
Each core runs, fully on-device, for its 32 token-sequences of length
257 (1 lang token + 256 vision latents, padded to 258 columns):

  lang-projection MLP + per-token vision projection  -> x  [1024, 32*258]
  4 pre-LN transformer encoder layers (H=4, dh=256)
  final LN + 5-layer box MLP on the fused (first) token -> [6, 32]

All matmuls run in float32r (fp32 rounded to 11 mantissa bits, processed
at bf16 rate by the PE; exact products, fp32 accumulation).  Activations
are kept feature-major ([d on partitions, tokens on free]); LayerNorm
statistics are computed with ones-matmuls on the PE and broadcast back
across partitions with rank-1 matmuls.  LN gamma/beta, attention scale
and the V bias are folded into weights on the host.  Layer 3 computes
attention only for the fused query column, and the final FF layer + box
head run on the compacted [1024, 32] fused matrix.
"""

import numpy as np

N_CORES = 8
T = 256
D_LM = 4096
D_VIS = 1024
D = 1024
L = 256
B = 16
FF = 1024
H = 4
NL = 4
EPS = 1e-5

TS = T // N_CORES      # tokens per core
S = 258                # padded sequence length (1 lang + 256 vis + 1 pad)
SR = 257               # real sequence length
F = TS * S             # flattened columns per core
DC = D // 128          # feature chunks (8)

_COMPILED = None


def _round_f32r(x):
    """Round float32 array to 11 mantissa bits (RNE) == float32r grid."""
    x = np.ascontiguousarray(x, dtype=np.float32)
    bits = x.view(np.uint32).astype(np.uint64)
    half = np.uint64(2047)
    lsb = (bits >> np.uint64(12)) & np.uint64(1)
    bits = (bits + half + lsb) >> np.uint64(12) << np.uint64(12)
    return bits.astype(np.uint32).view(np.float32)


def _bias_cols(b):
    """[n*128] bias vector -> [128, n] (column per 128-feature chunk)."""
    b = np.asarray(b, dtype=np.float32)
    return np.ascontiguousarray(b.reshape(-1, 128).T)


def _prep_weights(params):
    p = {k: np.asarray(v, dtype=np.float32) for k, v in params.items()}
    w = {}
    r = _round_f32r

    w["vis_WT"] = r(p["vis_W"].T)                      # [D_VIS, D]
    w["vis_b"] = _bias_cols(p["vis_b"])

    w["lang_W1T"] = r(p["lang_W1"].T)                  # [4096, 1024]
    w["lang_W2T"] = r(p["lang_W2"].T)
    w["lang_W3T"] = r(p["lang_W3"].T)
    w["lang_b1"] = _bias_cols(p["lang_b1"])
    w["lang_b2"] = _bias_cols(p["lang_b2"])
    w["lang_b3"] = _bias_cols(p["lang_b3"])

    for i in range(NL):
        g1 = p["enc_ln1_g"][i]
        be1 = p["enc_ln1_b"][i]
        Wqkv = p["enc_Wqkv"][i]
        bqkv = p["enc_bqkv"][i]
        Wq, Wk, Wv = Wqkv[0:D], Wqkv[D:2 * D], Wqkv[2 * D:3 * D]
        bq, bk, bv = bqkv[0:D], bqkv[D:2 * D], bqkv[2 * D:3 * D]

        Wq_f = Wq * g1[None, :]
        b_q = bq + Wq @ be1
        Wk_f = Wk * g1[None, :] / 16.0
        b_k = (bk + Wk @ be1) / 16.0
        Wv_f = Wv * g1[None, :]
        b_v = bv + Wv @ be1
        Wo = p["enc_Wo"][i]
        b_o = p["enc_bo"][i] + Wo @ b_v

        g2 = p["enc_ln2_g"][i]
        be2 = p["enc_ln2_b"][i]
        Wf1_f = p["enc_Wff1"][i] * g2[None, :]
        b_f1 = p["enc_bff1"][i] + p["enc_Wff1"][i] @ be2

        w[f"l{i}_WqT"] = r(Wq_f.T)
        w[f"l{i}_WkT"] = r(Wk_f.T)
        w[f"l{i}_WvT"] = r(Wv_f.T)
        w[f"l{i}_WoT"] = r(Wo.T)
        w[f"l{i}_bq"] = _bias_cols(b_q)
        w[f"l{i}_bk"] = _bias_cols(b_k)
        w[f"l{i}_bo"] = _bias_cols(b_o)
        w[f"l{i}_Wf1T"] = r(Wf1_f.T)
        w[f"l{i}_Wf2T"] = r(p["enc_Wff2"][i].T)
        w[f"l{i}_bf1"] = _bias_cols(b_f1)
        w[f"l{i}_bf2"] = _bias_cols(p["enc_bff2"][i])

    gf = p["enc_lnf_g"]
    bef = p["enc_lnf_b"]
    W1_f = p["box_W1"] * gf[None, :]
    b_1 = p["box_b1"] + p["box_W1"] @ bef
    w["box_W1T"] = r(W1_f.T)
    w["box_b1"] = _bias_cols(b_1)
    for j in (2, 3, 4):
        w[f"box_W{j}T"] = r(p[f"box_W{j}"].T)
        w[f"box_b{j}"] = _bias_cols(p[f"box_b{j}"])
    w["box_W5T"] = r(p["box_W5"].T)                    # [1024, 6]
    w["box_b5"] = np.ascontiguousarray(p["box_b5"].reshape(6, 1))

    w["consts"] = np.stack(
        [np.ones(128, np.float32), np.full(128, 1.0 / 1024.0, np.float32)], axis=1
    )
    w["ones_row"] = np.ones((1, 128), np.float32)
    w["zeros32"] = np.zeros((128, TS), np.float32)
    w["epsb"] = np.full((128, 1), EPS, np.float32)
    return w


# ---------------------------------------------------------------------------
# graph builder
# ---------------------------------------------------------------------------

def _build():
    import concourse.bacc as bacc
    import concourse.mybir as mybir
    from concourse.tile import TileContext
    import concourse.bass as bass

    f32 = mybir.dt.float32
    f32r = mybir.dt.float32r
    Alu = mybir.AluOpType
    Act = mybir.ActivationFunctionType

    nc = bacc.Bacc("TRN2", target_bir_lowering=False, debug=False,
                   num_devices=N_CORES)

    def din(name, shape, dtype=f32r):
        return nc.dram_tensor(name, list(shape), dtype, kind="ExternalInput")

    grdT = din("grdT", [D_LM, TS])
    visT = din("visT", [D_VIS, TS * L])
    consts = din("consts", [128, 2])
    ones_row_d = din("ones_row", [1, 128])
    zeros_d = din("zeros32", [128, TS])
    epsb_d = din("epsb", [128, 1], f32)
    vis_WT = din("vis_WT", [D_VIS, D])
    vis_b = din("vis_b", [128, DC], f32)
    lang_W1T = din("lang_W1T", [D_LM, FF])
    lang_W2T = din("lang_W2T", [FF, FF])
    lang_W3T = din("lang_W3T", [FF, D])
    lang_b = [din(f"lang_b{j}", [128, DC], f32) for j in (1, 2, 3)]
    lw = {}
    for i in range(NL):
        for nm in ("WqT", "WkT", "WvT", "WoT", "Wf1T", "Wf2T"):
            lw[(i, nm)] = din(f"l{i}_{nm}", [D, D])
        for nm in ("bq", "bk", "bo", "bf1", "bf2"):
            lw[(i, nm)] = din(f"l{i}_{nm}", [128, DC], f32)
    box_WT = {j: din(f"box_W{j}T", [D, D]) for j in (1, 2, 3, 4)}
    box_b = {j: din(f"box_b{j}", [128, DC], f32) for j in (1, 2, 3, 4)}
    box_W5T = din("box_W5T", [D, 6])
    box_b5 = din("box_b5", [6, 1], f32)

    out = nc.dram_tensor("out", [6, TS], f32, kind="ExternalOutput")

    ds = bass.ds

    with TileContext(nc) as tc:
        dram_cm = tc.tile_pool(name="dram", bufs=1, space="DRAM")
        dram = dram_cm.__enter__()
        xA = dram.tile([128, DC * F], f32r, name="xA")
        xB = dram.tile([128, DC * F], f32r, name="xB")
        xFz = dram.tile([128, DC * TS], f32r, name="xFz")   # fused cols after L3A

        cpool_cm = tc.tile_pool(name="cpool", bufs=1)
        cpool = cpool_cm.__enter__()
        ones2 = cpool.tile([128, 2], f32r)
        nc.sync.dma_start(out=ones2[:], in_=consts[:])
        ones_r = cpool.tile([1, 128], f32r)
        nc.sync.dma_start(out=ones_r[:], in_=ones_row_d[:])
        epsb = cpool.tile([128, 1], f32)
        nc.sync.dma_start(out=epsb[:], in_=epsb_d[:])

        # ---------------- helpers ----------------------------------------
        def layernorm(pool, pp, xs, n, tag):
            """feature-major LN over DC chunks of xs ([128, DC*n]); returns
            standardized xh [128, DC*n] f32r (gamma/beta folded into the
            following matmul's weights on the host)."""
            ps_mu = pp.tile([1, n], f32, tag=f"{tag}_mu", name=f"{tag}_mu")
            ps_s2 = pp.tile([1, n], f32, tag=f"{tag}_s2", name=f"{tag}_s2")
            for c in range(DC):
                nc.tensor.matmul(ps_mu[:], ones2[:, 1:2], xs[:, c * n:(c + 1) * n],
                                 start=(c == 0), stop=(c == DC - 1))
            for c in range(DC):
                xsq = pool.tile([128, n], f32r, tag=f"{tag}_xsq",
                                name=f"{tag}_xsq{c}", bufs=2)
                nc.scalar.square(xsq[:], xs[:, c * n:(c + 1) * n])
                nc.tensor.matmul(ps_s2[:], ones2[:, 1:2], xsq[:],
                                 start=(c == 0), stop=(c == DC - 1))
            mu = pool.tile([1, n], f32r, tag=f"{tag}_musb", name=f"{tag}_musb")
            nc.scalar.copy(mu[:], ps_mu[:])
            musq = pool.tile([1, n], f32, tag=f"{tag}_musq", name=f"{tag}_musq")
            nc.scalar.square(musq[:], mu[:])
            var = pool.tile([1, n], f32, tag=f"{tag}_var", name=f"{tag}_var")
            nc.vector.tensor_tensor(out=var[:], in0=ps_s2[:], in1=musq[:],
                                    op=Alu.subtract)
            sd = pool.tile([1, n], f32, tag=f"{tag}_sd", name=f"{tag}_sd")
            nc.scalar.activation(sd[:], var[:], Act.Sqrt, bias=epsb[0:1, :])
            rstd = pool.tile([1, n], f32r, tag=f"{tag}_rstd", name=f"{tag}_rstd")
            with nc.allow_low_precision(reason="f32r rstd for broadcast matmul"):
                nc.vector.reciprocal(rstd[:], sd[:])
            ps_mub = pp.tile([128, n], f32, tag=f"{tag}_mub", name=f"{tag}_mub")
            nc.tensor.matmul(ps_mub[:], ones_r[:], mu[:], start=True, stop=True)
            ps_rb = pp.tile([128, n], f32, tag=f"{tag}_rb", name=f"{tag}_rb")
            nc.tensor.matmul(ps_rb[:], ones_r[:], rstd[:], start=True, stop=True)
            xh = pool.tile([128, DC * n], f32r, tag=f"{tag}_xh", name=f"{tag}_xh")
            for c in range(DC):
                td = pool.tile([128, n], f32, tag=f"{tag}_td",
                               name=f"{tag}_td{c}", bufs=2)
                nc.vector.tensor_tensor(out=td[:], in0=xs[:, c * n:(c + 1) * n],
                                        in1=ps_mub[:], op=Alu.subtract)
                nc.vector.tensor_tensor(out=xh[:, c * n:(c + 1) * n], in0=td[:],
                                        in1=ps_rb[:], op=Alu.mult)
            return xh

        def mm_block(pp, pool, wtiles, rhs_tile, n, bias, act, out_tile, tag,
                     n_dc=DC, n_cc=DC):
            """out[:, cc*n:(cc+1)*n] =
            act(sum_dc wtiles[dc][:, cc*128:+128].T @ rhs[:, dc*n:+n] + bias[cc])"""
            for cc in range(n_cc):
                ps = pp.tile([128, n], f32, tag="acc", name=f"{tag}_ps{cc}", bufs=2)
                for dc in range(n_dc):
                    nc.tensor.matmul(ps[:], wtiles[dc][:, cc * 128:(cc + 1) * 128],
                                     rhs_tile[:, dc * n:(dc + 1) * n],
                                     start=(dc == 0), stop=(dc == n_dc - 1))
                dst = out_tile[:, cc * n:(cc + 1) * n]
                if act == "relu":
                    nc.scalar.activation(dst, ps[:], Act.Relu, bias=bias[:, cc:cc + 1])
                elif act == "bias":
                    nc.vector.tensor_scalar_add(out=dst, in0=ps[:],
                                                scalar1=bias[:, cc:cc + 1])
                else:
                    nc.vector.tensor_copy(out=dst, in_=ps[:])

        class _WView:
            def __init__(self, tile, ncols):
                self.tile = tile
                self.ncols = ncols
            def __getitem__(self, dcv):
                return self.tile[:, dcv * self.ncols:(dcv + 1) * self.ncols]

        def load_w(pool, dram_t, name, ncols=D):
            t = pool.tile([128, DC * ncols], f32r, tag=name, name=name)
            nc.sync.dma_start(
                out=t[:].rearrange("p (c f) -> p c f", c=DC),
                in_=dram_t[:].rearrange("(c p) f -> p c f", p=128))
            return _WView(t, ncols)

        def load_b(pool, dram_t, name):
            t = pool.tile([128, DC], f32, name=name)
            nc.sync.dma_start(out=t[:], in_=dram_t[:])
            return t

        # ---------------- stage 0: lang MLP ------------------------------
        with tc.tile_pool(name="lang", bufs=1) as pool, \
             tc.tile_pool(name="lang_ps", bufs=1, space="PSUM") as pp:
            g_sb = pool.tile([128, 32 * TS], f32r)
            nc.sync.dma_start(
                out=g_sb[:].rearrange("p (c f) -> p c f", c=32),
                in_=grdT[:].rearrange("(c p) f -> p c f", p=128))
            w1t = pool.tile([128, 32 * FF], f32r, name="lw1")
            nc.sync.dma_start(
                out=w1t[:].rearrange("p (c f) -> p c f", c=32),
                in_=lang_W1T[:].rearrange("(c p) f -> p c f", p=128))
            w1 = _WView(w1t, FF)
            b1 = load_b(pool, lang_b[0], "lb1")
            b2 = load_b(pool, lang_b[1], "lb2")
            b3 = load_b(pool, lang_b[2], "lb3")
            h1 = pool.tile([128, DC * TS], f32r)
            mm_block(pp, pool, w1, g_sb, TS, b1, "relu", h1, "lg1", n_dc=32)
            w2 = load_w(pool, lang_W2T, "lw2", FF)
            h2 = pool.tile([128, DC * TS], f32r)
            mm_block(pp, pool, w2, h1, TS, b2, "relu", h2, "lg2")
            w3 = load_w(pool, lang_W3T, "lw3", D)
            lang_o = pool.tile([128, DC * TS], f32r)
            mm_block(pp, pool, w3, h2, TS, b3, "bias", lang_o, "lg3")
            zt = pool.tile([128, TS], f32r)
            nc.sync.dma_start(out=zt[:], in_=zeros_d[:])
            for c in range(DC):
                xa_c = xA[:, c * F:(c + 1) * F].rearrange("p (t s) -> p t s", s=S)
                nc.sync.dma_start(out=xa_c[:, :, 0:1],
                                  in_=lang_o[:, c * TS:(c + 1) * TS])
                nc.sync.dma_start(out=xa_c[:, :, SR:S], in_=zt[:])

        # ---------------- stage 1: vision projection ---------------------
        with tc.tile_pool(name="vis", bufs=1) as pool, \
             tc.tile_pool(name="vis_ps", bufs=1, space="PSUM") as pp:
            wv = load_w(pool, vis_WT, "visw", D)
            vb = load_b(pool, vis_b, "visb")
            with tc.For_i(0, TS, hint_engines=(mybir.EngineType.PE,)) as iv:
                v_in = pool.tile([128, DC * L], f32r, tag="v_in", bufs=2)
                for c in range(DC):
                    nc.sync.dma_start(out=v_in[:, c * L:(c + 1) * L],
                                      in_=visT[c * 128:(c + 1) * 128, ds(iv * L, L)])
                v_out = pool.tile([128, DC * L], f32r, tag="v_out", bufs=2)
                mm_block(pp, pool, wv, v_in, L, vb, "bias", v_out, "vis")
                for c in range(DC):
                    nc.sync.dma_start(out=xA[:, ds(c * F + iv * S + 1, L)],
                                      in_=v_out[:, c * L:(c + 1) * L])

        # ---------------- encoder layers ---------------------------------
        def attn_pass(li, xin, xout, trim):
            NQ = 2 if trim else S
            with tc.tile_pool(name=f"l{li}a", bufs=1) as pool, \
                 tc.tile_pool(name=f"l{li}a_ps", bufs=1, space="PSUM") as pp:
                wq = load_w(pool, lw[(li, "WqT")], f"wq{li}")
                wk = load_w(pool, lw[(li, "WkT")], f"wk{li}")
                wv_ = load_w(pool, lw[(li, "WvT")], f"wv{li}")
                wo = load_w(pool, lw[(li, "WoT")], f"wo{li}")
                bq_s = load_b(pool, lw[(li, "bq")], f"bq{li}")
                bk_s = load_b(pool, lw[(li, "bk")], f"bk{li}")
                bo_s = load_b(pool, lw[(li, "bo")], f"bo{li}")

                with tc.For_i(0, TS, hint_engines=(mybir.EngineType.PE,)) as iv:
                    xs = pool.tile([128, DC * S], f32r, tag="xs", bufs=1)
                    nc.sync.dma_start(
                        out=xs[:].rearrange("p (c f) -> p c f", c=DC),
                        in_=xin[:].rearrange("p (c f) -> p c f", c=DC)[:, :, ds(iv * S, S)])
                    xh = layernorm(pool, pp, xs, S, f"ln{li}a")

                    k_sb = pool.tile([128, DC * S], f32r, tag="k_sb")
                    mm_block(pp, pool, wk, xh, S, bk_s, "bias", k_sb, "k")
                    q_sb = pool.tile([128, DC * NQ], f32r, tag="q_sb")
                    if trim:
                        xhq = pool.tile([128, DC * NQ], f32r, tag="xhq")
                        for c in range(DC):
                            nc.vector.tensor_copy(out=xhq[:, c * NQ:(c + 1) * NQ],
                                                  in_=xh[:, c * S:c * S + NQ])
                        mm_block(pp, pool, wq, xhq, NQ, bq_s, "bias", q_sb, "q")
                    else:
                        mm_block(pp, pool, wq, xh, S, bq_s, "bias", q_sb, "q")

                    # V token-major: rows = sequence positions, cols = 4*256
                    v_sb = [pool.tile([128, D], f32r, tag=f"v_sb{fc}",
                                      name=f"v_sb{fc}") for fc in range(3)]
                    for fc, (f0, fn) in enumerate(((0, 128), (128, 128), (256, 2))):
                        for hp in range(2):
                            ps = pp.tile([128, 512], f32, tag="acc",
                                         name=f"vps{fc}_{hp}", bufs=2)
                            for c in range(DC):
                                nc.tensor.matmul(
                                    ps[0:fn, :],
                                    xh[:, c * S + f0:c * S + f0 + fn],
                                    wv_[c][:, hp * 512:(hp + 1) * 512],
                                    start=(c == 0), stop=(c == DC - 1))
                            nc.vector.tensor_copy(
                                out=v_sb[fc][0:fn, hp * 512:(hp + 1) * 512],
                                in_=ps[0:fn, :])

                    on_sb = pool.tile([128, DC * NQ], f32r, tag="on_sb")
                    kchunks = ((0, 128), (128, 128), (256, 1))
                    for h in range(H):
                        es = pool.tile([128, 2 * NQ], f32r, tag="es",
                                       name=f"es{h}", bufs=2)
                        es2 = pool.tile([1, NQ], f32r, tag="es2",
                                        name=f"es2{h}", bufs=2)
                        for kc, (k0, kn) in enumerate(kchunks):
                            ps_st = pp.tile([128, NQ], f32, tag="acc",
                                            name=f"st{h}_{kc}", bufs=2)
                            for d2 in range(2):
                                kt = 2 * h + d2
                                nc.tensor.matmul(
                                    ps_st[0:kn, :],
                                    k_sb[:, kt * S + k0:kt * S + k0 + kn],
                                    q_sb[:, kt * NQ:(kt + 1) * NQ],
                                    start=(d2 == 0), stop=(d2 == 1))
                            tgt = es[0:kn, kc * NQ:(kc + 1) * NQ] if kc < 2 \
                                else es2[0:1, :]
                            nc.scalar.activation(tgt, ps_st[0:kn, :], Act.Exp)
                        ps_den = pp.tile([1, NQ], f32, tag="den", name=f"den{h}")
                        nc.tensor.matmul(ps_den[:], ones2[:, 0:1],
                                         es[:, 0:NQ], start=True, stop=False)
                        nc.tensor.matmul(ps_den[:], ones2[:, 0:1],
                                         es[:, NQ:2 * NQ], start=False, stop=False)
                        nc.tensor.matmul(ps_den[:], ones2[0:1, 0:1],
                                         es2[0:1, :], start=False, stop=True)
                        recip = pool.tile([1, NQ], f32r, tag="recip",
                                          name=f"recip{h}")
                        with nc.allow_low_precision(reason="softmax denom recip"):
                            nc.vector.reciprocal(recip[:], ps_den[:])
                        ps_rb = pp.tile([128, NQ], f32, tag="rbb", name=f"rb{h}")
                        nc.tensor.matmul(ps_rb[:], ones_r[:], recip[:],
                                         start=True, stop=True)
                        rb_sb = pool.tile([128, NQ], f32, tag="rb_sb",
                                          name=f"rb_sb{h}")
                        nc.scalar.copy(rb_sb[:], ps_rb[:])
                        for d2 in range(2):
                            ps_o = pp.tile([128, NQ], f32, tag="acc",
                                           name=f"o{h}_{d2}", bufs=2)
                            for kc, (k0, kn) in enumerate(kchunks):
                                src = es[0:kn, kc * NQ:(kc + 1) * NQ] if kc < 2 \
                                    else es2[0:1, :]
                                nc.tensor.matmul(
                                    ps_o[:],
                                    v_sb[kc][0:kn, h * 256 + d2 * 128:
                                             h * 256 + d2 * 128 + 128],
                                    src, start=(kc == 0), stop=(kc == 2))
                            cc = 2 * h + d2
                            nc.vector.tensor_tensor(
                                out=on_sb[:, cc * NQ:(cc + 1) * NQ],
                                in0=ps_o[:], in1=rb_sb[:], op=Alu.mult)

                    for cc in range(DC):
                        ps = pp.tile([128, NQ], f32, tag="acc",
                                     name=f"wops{cc}", bufs=2)
                        for dcv in range(DC):
                            nc.tensor.matmul(ps[:],
                                             wo[dcv][:, cc * 128:(cc + 1) * 128],
                                             on_sb[:, dcv * NQ:(dcv + 1) * NQ],
                                             start=(dcv == 0), stop=(dcv == DC - 1))
                        xnew = pool.tile([128, NQ], f32r, tag="xnew",
                                         name=f"xnew{cc}", bufs=3)
                        nc.vector.scalar_tensor_tensor(
                            out=xnew[:], in0=ps[:], scalar=bo_s[:, cc:cc + 1],
                            in1=xs[:, cc * S:cc * S + NQ],
                            op0=Alu.add, op1=Alu.add)
                        if trim:
                            nc.sync.dma_start(out=xFz[:, ds(cc * TS + iv, 1)],
                                              in_=xnew[:, 0:1])
                        else:
                            nc.sync.dma_start(out=xout[:, ds(cc * F + iv * S, S)],
                                              in_=xnew[:])

        def ff_pass(li, xin, xout):
            with tc.tile_pool(name=f"l{li}b", bufs=1) as pool, \
                 tc.tile_pool(name=f"l{li}b_ps", bufs=1, space="PSUM") as pp:
                wf1 = load_w(pool, lw[(li, "Wf1T")], f"wf1{li}")
                wf2 = load_w(pool, lw[(li, "Wf2T")], f"wf2{li}")
                bf1_s = load_b(pool, lw[(li, "bf1")], f"bf1{li}")
                bf2_s = load_b(pool, lw[(li, "bf2")], f"bf2{li}")
                with tc.For_i(0, TS, hint_engines=(mybir.EngineType.PE,)) as iv:
                    xs = pool.tile([128, DC * S], f32r, tag="xs", bufs=2)
                    nc.sync.dma_start(
                        out=xs[:].rearrange("p (c f) -> p c f", c=DC),
                        in_=xin[:].rearrange("p (c f) -> p c f", c=DC)[:, :, ds(iv * S, S)])
                    xh = layernorm(pool, pp, xs, S, f"ln{li}b")
                    hmid = pool.tile([128, DC * S], f32r, tag="hmid")
                    mm_block(pp, pool, wf1, xh, S, bf1_s, "relu", hmid, "ff1")
                    for cc in range(DC):
                        ps = pp.tile([128, S], f32, tag="acc",
                                     name=f"f2ps{cc}", bufs=2)
                        for dcv in range(DC):
                            nc.tensor.matmul(ps[:],
                                             wf2[dcv][:, cc * 128:(cc + 1) * 128],
                                             hmid[:, dcv * S:(dcv + 1) * S],
                                             start=(dcv == 0), stop=(dcv == DC - 1))
                        xnew = pool.tile([128, S], f32r, tag="xnew",
                                         name=f"fxnew{cc}", bufs=3)
                        nc.vector.scalar_tensor_tensor(
                            out=xnew[:], in0=ps[:], scalar=bf2_s[:, cc:cc + 1],
                            in1=xs[:, cc * S:(cc + 1) * S],
                            op0=Alu.add, op1=Alu.add)
                        nc.sync.dma_start(out=xout[:, ds(cc * F + iv * S, S)],
                                          in_=xnew[:])

        attn_pass(0, xA, xB, trim=False)
        ff_pass(0, xB, xA)
        attn_pass(1, xA, xB, trim=False)
        ff_pass(1, xB, xA)
        attn_pass(2, xA, xB, trim=False)
        ff_pass(2, xB, xA)
        attn_pass(3, xA, None, trim=True)

        # ---------------- final FF + LN + box head on fused cols ---------
        with tc.tile_pool(name="post", bufs=1) as pool, \
             tc.tile_pool(name="post_ps", bufs=1, space="PSUM") as pp:
            xs = pool.tile([128, DC * TS], f32r)
            nc.sync.dma_start(out=xs[:], in_=xFz[:])
            wf1 = load_w(pool, lw[(3, "Wf1T")], "wf13")
            wf2 = load_w(pool, lw[(3, "Wf2T")], "wf23")
            bf1_s = load_b(pool, lw[(3, "bf1")], "bf13")
            bf2_s = load_b(pool, lw[(3, "bf2")], "bf23")
            xh = layernorm(pool, pp, xs, TS, "lnp")
            hmid = pool.tile([128, DC * TS], f32r)
            mm_block(pp, pool, wf1, xh, TS, bf1_s, "relu", hmid, "pf1")
            xfin = pool.tile([128, DC * TS], f32r)
            for cc in range(DC):
                ps = pp.tile([128, TS], f32, tag="acc", name=f"pf2ps{cc}", bufs=2)
                for dcv in range(DC):
                    nc.tensor.matmul(ps[:], wf2[dcv][:, cc * 128:(cc + 1) * 128],
                                     hmid[:, dcv * TS:(dcv + 1) * TS],
                                     start=(dcv == 0), stop=(dcv == DC - 1))
                nc.vector.scalar_tensor_tensor(
                    out=xfin[:, cc * TS:(cc + 1) * TS], in0=ps[:],
                    scalar=bf2_s[:, cc:cc + 1], in1=xs[:, cc * TS:(cc + 1) * TS],
                    op0=Alu.add, op1=Alu.add)
            xhf = layernorm(pool, pp, xfin, TS, "lnp")
            cur = xhf
            for j in (1, 2, 3, 4):
                wj = load_w(pool, box_WT[j], f"bx{j}")
                bj_s = load_b(pool, box_b[j], f"bxb{j}")
                nxt = pool.tile([128, DC * TS], f32r, name=f"bxh{j}")
                mm_block(pp, pool, wj, cur, TS, bj_s, "relu", nxt, f"bx{j}")
                cur = nxt
            w5 = pool.tile([128, DC * 6], f32r)
            nc.sync.dma_start(
                out=w5[:].rearrange("p (c f) -> p c f", c=DC),
                in_=box_W5T[:].rearrange("(c p) f -> p c f", p=128))
            b5_s = pool.tile([6, 1], f32)
            nc.sync.dma_start(out=b5_s[:], in_=box_b5[:])
            ps5 = pp.tile([6, TS], f32, name="ps5")
            for dcv in range(DC):
                nc.tensor.matmul(ps5[:], w5[:, dcv * 6:(dcv + 1) * 6],
                                 cur[:, dcv * TS:(dcv + 1) * TS],
                                 start=(dcv == 0), stop=(dcv == DC - 1))
            ob = pool.tile([6, TS], f32)
            nc.vector.tensor_scalar_add(out=ob[:], in0=ps5[:], scalar1=b5_s[:])
            nc.sync.dma_start(out=out[:], in_=ob[:])

        cpool_cm.__exit__(None, None, None)
        dram_cm.__exit__(None, None, None)

    nc.compile()
    return nc


def _get_compiled():
    global _COMPILED
    if _COMPILED is None:
        _COMPILED = _build()
    return _COMPILED


# ---------------------------------------------------------------------------
# host entry point
# ---------------------------------------------------------------------------

def kernel(grd_token_hidden_states, batch_idx, vision_features_before_mm_projection,
           params):
    from concourse.bass_utils import run_bass_kernel_spmd

    grd = np.asarray(grd_token_hidden_states, dtype=np.float32)
    bidx = np.asarray(batch_idx).astype(np.int64)
    vis = np.asarray(vision_features_before_mm_projection, dtype=np.float32)

    w = _prep_weights(params)
    nc = _get_compiled()

    in_maps = []
    for i in range(N_CORES):
        sl = slice(i * TS, (i + 1) * TS)
        m = dict(w)
        m["grdT"] = _round_f32r(grd[sl].T)
        gathered = vis[bidx[sl]]                      # [TS, L, D_VIS]
        m["visT"] = _round_f32r(
            gathered.transpose(2, 0, 1).reshape(D_VIS, TS * L))
        in_maps.append(m)

    res = run_bass_kernel_spmd(nc, in_maps, core_ids=list(range(N_CORES)))
    outs = [np.asarray(res.results[i]["out"]).T for i in range(N_CORES)]
    return np.concatenate(outs, axis=0).astype(np.float32)


# revision 8
# speedup vs baseline: 1.0271x; 1.0152x over previous
"""Trainium2 Bass kernel for BBoxHeadForGroundTruthBboxRegressionV2.

Strategy
--------
Data-parallel over the 256 ground tokens: 8 NeuronCores x 32 tokens.
Each core runs, fully on-device, for its 32 token-sequences of length
257 (1 lang token + 256 vision latents, padded to 258 columns):

  lang-projection MLP + per-token vision projection  -> x  [1024, 32*258]
  4 pre-LN transformer encoder layers (H=4, dh=256)
  final LN + 5-layer box MLP on the fused (first) token -> [6, 32]

All matmuls run in float32r (fp32 rounded to 11 mantissa bits, processed
at bf16 rate by the PE; exact products, fp32 accumulation).  Activations
are kept feature-major ([d on partitions, tokens on free]); LayerNorm
statistics are computed with ones-matmuls on the PE and broadcast back
across partitions with rank-1 matmuls.  LN gamma/beta, attention scale
and the V bias are folded into weights on the host.  Layer 3 computes
attention only for the fused query column, and the final FF layer + box
head run on the compacted [1024, 32] fused matrix.
"""

import numpy as np

N_CORES = 8
T = 256
D_LM = 4096
D_VIS = 1024
D = 1024
L = 256
B = 16
FF = 1024
H = 4
NL = 4
EPS = 1e-5

TS = T // N_CORES      # tokens per core
S = 258                # padded sequence length (1 lang + 256 vis + 1 pad)
SR = 257               # real sequence length
F = TS * S             # flattened columns per core
DC = D // 128          # feature chunks (8)

_COMPILED = None


def _round_f32r(x):
    """Round float32 array to 11 mantissa bits (RNE) == float32r grid."""
    x = np.ascontiguousarray(x, dtype=np.float32)
    bits = x.view(np.uint32).astype(np.uint64)
    half = np.uint64(2047)
    lsb = (bits >> np.uint64(12)) & np.uint64(1)
    bits = (bits + half + lsb) >> np.uint64(12) << np.uint64(12)
    return bits.astype(np.uint32).view(np.float32)


def _bias_cols(b):
    """[n*128] bias vector -> [128, n] (column per 128-feature chunk)."""
    b = np.asarray(b, dtype=np.float32)
    return np.ascontiguousarray(b.reshape(-1, 128).T)


def _prep_weights(params):
    p = {k: np.asarray(v, dtype=np.float32) for k, v in params.items()}
    w = {}
    r = _round_f32r

    w["vis_WT"] = r(p["vis_W"].T)                      # [D_VIS, D]
    w["vis_b"] = _bias_cols(p["vis_b"])

    w["lang_W1T"] = r(p["lang_W1"].T)                  # [4096, 1024]
    w["lang_W2T"] = r(p["lang_W2"].T)
    w["lang_W3T"] = r(p["lang_W3"].T)
    w["lang_b1"] = _bias_cols(p["lang_b1"])
    w["lang_b2"] = _bias_cols(p["lang_b2"])
    w["lang_b3"] = _bias_cols(p["lang_b3"])

    for i in range(NL):
        g1 = p["enc_ln1_g"][i]
        be1 = p["enc_ln1_b"][i]
        Wqkv = p["enc_Wqkv"][i]
        bqkv = p["enc_bqkv"][i]
        Wq, Wk, Wv = Wqkv[0:D], Wqkv[D:2 * D], Wqkv[2 * D:3 * D]
        bq, bk, bv = bqkv[0:D], bqkv[D:2 * D], bqkv[2 * D:3 * D]

        Wq_f = Wq * g1[None, :]
        b_q = bq + Wq @ be1
        Wk_f = Wk * g1[None, :] / 16.0
        b_k = (bk + Wk @ be1) / 16.0
        Wv_f = Wv * g1[None, :]
        b_v = bv + Wv @ be1
        Wo = p["enc_Wo"][i]
        b_o = p["enc_bo"][i] + Wo @ b_v

        g2 = p["enc_ln2_g"][i]
        be2 = p["enc_ln2_b"][i]
        Wf1_f = p["enc_Wff1"][i] * g2[None, :]
        b_f1 = p["enc_bff1"][i] + p["enc_Wff1"][i] @ be2

        w[f"l{i}_WqT"] = r(Wq_f.T)
        w[f"l{i}_WkT"] = r(Wk_f.T)
        w[f"l{i}_WvT"] = r(Wv_f.T)
        w[f"l{i}_WoT"] = r(Wo.T)
        w[f"l{i}_bq"] = _bias_cols(b_q)
        w[f"l{i}_bk"] = _bias_cols(b_k)
        w[f"l{i}_bo"] = _bias_cols(b_o)
        w[f"l{i}_Wf1T"] = r(Wf1_f.T)
        w[f"l{i}_Wf2T"] = r(p["enc_Wff2"][i].T)
        w[f"l{i}_bf1"] = _bias_cols(b_f1)
        w[f"l{i}_bf2"] = _bias_cols(p["enc_bff2"][i])

    gf = p["enc_lnf_g"]
    bef = p["enc_lnf_b"]
    W1_f = p["box_W1"] * gf[None, :]
    b_1 = p["box_b1"] + p["box_W1"] @ bef
    w["box_W1T"] = r(W1_f.T)
    w["box_b1"] = _bias_cols(b_1)
    for j in (2, 3, 4):
        w[f"box_W{j}T"] = r(p[f"box_W{j}"].T)
        w[f"box_b{j}"] = _bias_cols(p[f"box_b{j}"])
    w["box_W5T"] = r(p["box_W5"].T)                    # [1024, 6]
    w["box_b5"] = np.ascontiguousarray(p["box_b5"].reshape(6, 1))

    w["consts"] = np.stack(
        [np.ones(128, np.float32), np.full(128, 1.0 / 1024.0, np.float32)], axis=1
    )
    w["ones_row"] = np.ones((1, 128), np.float32)
    w["zeros32"] = np.zeros((128, TS), np.float32)
    w["epsb"] = np.full((128, 1), EPS, np.float32)
    return w


# ---------------------------------------------------------------------------
# graph builder
# ---------------------------------------------------------------------------

def _build():
    import concourse.bacc as bacc
    import concourse.mybir as mybir
    from concourse.tile import TileContext
    import concourse.bass as bass

    f32 = mybir.dt.float32
    f32r = mybir.dt.float32r
    Alu = mybir.AluOpType
    Act = mybir.ActivationFunctionType

    nc = bacc.Bacc("TRN2", target_bir_lowering=False, debug=False,
                   num_devices=N_CORES)

    def din(name, shape, dtype=f32r):
        return nc.dram_tensor(name, list(shape), dtype, kind="ExternalInput")

    grdT = din("grdT", [D_LM, TS])
    visT = din("visT", [D_VIS, TS * L])
    consts = din("consts", [128, 2])
    ones_row_d = din("ones_row", [1, 128])
    zeros_d = din("zeros32", [128, TS])
    epsb_d = din("epsb", [128, 1], f32)
    vis_WT = din("vis_WT", [D_VIS, D])
    vis_b = din("vis_b", [128, DC], f32)
    lang_W1T = din("lang_W1T", [D_LM, FF])
    lang_W2T = din("lang_W2T", [FF, FF])
    lang_W3T = din("lang_W3T", [FF, D])
    lang_b = [din(f"lang_b{j}", [128, DC], f32) for j in (1, 2, 3)]
    lw = {}
    for i in range(NL):
        for nm in ("WqT", "WkT", "WvT", "WoT", "Wf1T", "Wf2T"):
            lw[(i, nm)] = din(f"l{i}_{nm}", [D, D])
        for nm in ("bq", "bk", "bo", "bf1", "bf2"):
            lw[(i, nm)] = din(f"l{i}_{nm}", [128, DC], f32)
    box_WT = {j: din(f"box_W{j}T", [D, D]) for j in (1, 2, 3, 4)}
    box_b = {j: din(f"box_b{j}", [128, DC], f32) for j in (1, 2, 3, 4)}
    box_W5T = din("box_W5T", [D, 6])
    box_b5 = din("box_b5", [6, 1], f32)

    out = nc.dram_tensor("out", [6, TS], f32, kind="ExternalOutput")

    ds = bass.ds

    with TileContext(nc) as tc:
        dram_cm = tc.tile_pool(name="dram", bufs=1, space="DRAM")
        dram = dram_cm.__enter__()
        xA = dram.tile([128, DC * F], f32r, name="xA")
        xB = dram.tile([128, DC * F], f32r, name="xB")
        xFz = dram.tile([128, DC * TS], f32r, name="xFz")   # fused cols after L3A

        cpool_cm = tc.tile_pool(name="cpool", bufs=1)
        cpool = cpool_cm.__enter__()
        ones2 = cpool.tile([128, 2], f32r)
        nc.sync.dma_start(out=ones2[:], in_=consts[:])
        ones_r = cpool.tile([1, 128], f32r)
        nc.sync.dma_start(out=ones_r[:], in_=ones_row_d[:])
        epsb = cpool.tile([128, 1], f32)
        nc.sync.dma_start(out=epsb[:], in_=epsb_d[:])

        # ---------------- helpers ----------------------------------------
        def layernorm(pool, pp, xs, n, tag):
            """feature-major LN over DC chunks of xs ([128, DC*n]); returns
            standardized xh [128, DC*n] f32r (gamma/beta folded into the
            following matmul's weights on the host)."""
            ps_mu = pp.tile([1, n], f32, tag=f"{tag}_mu", name=f"{tag}_mu")
            ps_s2 = pp.tile([1, n], f32, tag=f"{tag}_s2", name=f"{tag}_s2")
            for c in range(DC):
                nc.tensor.matmul(ps_mu[:], ones2[:, 1:2], xs[:, c * n:(c + 1) * n],
                                 start=(c == 0), stop=(c == DC - 1))
            for c in range(DC):
                xsq = pool.tile([128, n], f32r, tag=f"{tag}_xsq",
                                name=f"{tag}_xsq{c}", bufs=2)
                nc.scalar.square(xsq[:], xs[:, c * n:(c + 1) * n])
                nc.tensor.matmul(ps_s2[:], ones2[:, 1:2], xsq[:],
                                 start=(c == 0), stop=(c == DC - 1))
            mu = pool.tile([1, n], f32r, tag=f"{tag}_musb", name=f"{tag}_musb")
            nc.scalar.copy(mu[:], ps_mu[:])
            musq = pool.tile([1, n], f32, tag=f"{tag}_musq", name=f"{tag}_musq")
            nc.scalar.square(musq[:], mu[:])
            var = pool.tile([1, n], f32, tag=f"{tag}_var", name=f"{tag}_var")
            nc.vector.tensor_tensor(out=var[:], in0=ps_s2[:], in1=musq[:],
                                    op=Alu.subtract)
            sd = pool.tile([1, n], f32, tag=f"{tag}_sd", name=f"{tag}_sd")
            nc.scalar.activation(sd[:], var[:], Act.Sqrt, bias=epsb[0:1, :])
            rstd = pool.tile([1, n], f32r, tag=f"{tag}_rstd", name=f"{tag}_rstd")
            with nc.allow_low_precision(reason="f32r rstd for broadcast matmul"):
                nc.vector.reciprocal(rstd[:], sd[:])
            ps_mub = pp.tile([128, n], f32, tag=f"{tag}_mub", name=f"{tag}_mub")
            nc.tensor.matmul(ps_mub[:], ones_r[:], mu[:], start=True, stop=True)
            ps_rb = pp.tile([128, n], f32, tag=f"{tag}_rb", name=f"{tag}_rb")
            nc.tensor.matmul(ps_rb[:], ones_r[:], rstd[:], start=True, stop=True)
            xh = pool.tile([128, DC * n], f32r, tag=f"{tag}_xh", name=f"{tag}_xh")
            for c in range(DC):
                td = pool.tile([128, n], f32, tag=f"{tag}_td",
                               name=f"{tag}_td{c}", bufs=2)
                nc.vector.tensor_tensor(out=td[:], in0=xs[:, c * n:(c + 1) * n],
                                        in1=ps_mub[:], op=Alu.subtract)
                nc.vector.tensor_tensor(out=xh[:, c * n:(c + 1) * n], in0=td[:],
                                        in1=ps_rb[:], op=Alu.mult)
            return xh

        def mm_block(pp, pool, wtiles, rhs_tile, n, bias, act, out_tile, tag,
                     n_dc=DC, n_cc=DC):
            """out[:, cc*n:(cc+1)*n] =
            act(sum_dc wtiles[dc][:, cc*128:+128].T @ rhs[:, dc*n:+n] + bias[cc])"""
            for cc in range(n_cc):
                ps = pp.tile([128, n], f32, tag="acc", name=f"{tag}_ps{cc}", bufs=2)
                for dc in range(n_dc):
                    nc.tensor.matmul(ps[:], wtiles[dc][:, cc * 128:(cc + 1) * 128],
                                     rhs_tile[:, dc * n:(dc + 1) * n],
                                     start=(dc == 0), stop=(dc == n_dc - 1))
                dst = out_tile[:, cc * n:(cc + 1) * n]
                if act == "relu":
                    nc.scalar.activation(dst, ps[:], Act.Relu, bias=bias[:, cc:cc + 1])
                elif act == "bias":
                    nc.vector.tensor_scalar_add(out=dst, in0=ps[:],
                                                scalar1=bias[:, cc:cc + 1])
                else:
                    nc.vector.tensor_copy(out=dst, in_=ps[:])

        class _WView:
            def __init__(self, tile, ncols):
                self.tile = tile
                self.ncols = ncols
            def __getitem__(self, dcv):
                return self.tile[:, dcv * self.ncols:(dcv + 1) * self.ncols]

        def load_w(pool, dram_t, name, ncols=D):
            t = pool.tile([128, DC * ncols], f32r, tag=name, name=name)
            tv = t[:].rearrange("p (c f) -> p c f", c=DC)
            dv = dram_t[:].rearrange("(c p) f -> p c f", p=128)
            for q in range(4):
                nc.sync.dma_start(out=tv[:, 2 * q:2 * q + 2, :],
                                  in_=dv[:, 2 * q:2 * q + 2, :])
            return _WView(t, ncols)

        def load_b(pool, dram_t, name):
            t = pool.tile([128, DC], f32, name=name)
            nc.sync.dma_start(out=t[:], in_=dram_t[:])
            return t

        # ---------------- stage 0: lang MLP ------------------------------
        with tc.tile_pool(name="lang", bufs=1) as pool, \
             tc.tile_pool(name="lang_ps", bufs=1, space="PSUM") as pp:
            g_sb = pool.tile([128, 32 * TS], f32r)
            nc.sync.dma_start(
                out=g_sb[:].rearrange("p (c f) -> p c f", c=32),
                in_=grdT[:].rearrange("(c p) f -> p c f", p=128))
            w1t = pool.tile([128, 32 * FF], f32r, name="lw1")
            w1v = w1t[:].rearrange("p (c f) -> p c f", c=32)
            l1v = lang_W1T[:].rearrange("(c p) f -> p c f", p=128)
            for q in range(8):
                nc.sync.dma_start(out=w1v[:, 4 * q:4 * q + 4, :],
                                  in_=l1v[:, 4 * q:4 * q + 4, :])
            w1 = _WView(w1t, FF)
            b1 = load_b(pool, lang_b[0], "lb1")
            b2 = load_b(pool, lang_b[1], "lb2")
            b3 = load_b(pool, lang_b[2], "lb3")
            h1 = pool.tile([128, DC * TS], f32r)
            mm_block(pp, pool, w1, g_sb, TS, b1, "relu", h1, "lg1", n_dc=32)
            w2 = load_w(pool, lang_W2T, "lw2", FF)
            h2 = pool.tile([128, DC * TS], f32r)
            mm_block(pp, pool, w2, h1, TS, b2, "relu", h2, "lg2")
            w3 = load_w(pool, lang_W3T, "lw3", D)
            lang_o = pool.tile([128, DC * TS], f32r)
            mm_block(pp, pool, w3, h2, TS, b3, "bias", lang_o, "lg3")
            zt = pool.tile([128, TS], f32r)
            nc.sync.dma_start(out=zt[:], in_=zeros_d[:])
            for c in range(DC):
                xa_c = xA[:, c * F:(c + 1) * F].rearrange("p (t s) -> p t s", s=S)
                nc.sync.dma_start(out=xa_c[:, :, 0:1],
                                  in_=lang_o[:, c * TS:(c + 1) * TS])
                nc.sync.dma_start(out=xa_c[:, :, SR:S], in_=zt[:])

        # ---------------- stage 1: vision projection ---------------------
        with tc.tile_pool(name="vis", bufs=1) as pool, \
             tc.tile_pool(name="vis_ps", bufs=1, space="PSUM") as pp:
            wv = load_w(pool, vis_WT, "visw", D)
            vb = load_b(pool, vis_b, "visb")
            with tc.For_i(0, TS, hint_engines=(mybir.EngineType.PE,), staggered_reset=True) as iv:
                v_in = pool.tile([128, DC * L], f32r, tag="v_in", bufs=2)
                for c in range(DC):
                    nc.sync.dma_start(out=v_in[:, c * L:(c + 1) * L],
                                      in_=visT[c * 128:(c + 1) * 128, ds(iv * L, L)])
                v_out = pool.tile([128, DC * L], f32r, tag="v_out", bufs=2)
                mm_block(pp, pool, wv, v_in, L, vb, "bias", v_out, "vis")
                for c in range(DC):
                    nc.sync.dma_start(out=xA[:, ds(c * F + iv * S + 1, L)],
                                      in_=v_out[:, c * L:(c + 1) * L])

        # ---------------- encoder layers ---------------------------------
        def attn_pass(li, xin, xout, trim):
            NQ = 2 if trim else S
            with tc.tile_pool(name=f"l{li}a", bufs=1) as pool, \
                 tc.tile_pool(name=f"l{li}a_ps", bufs=1, space="PSUM") as pp:
                wq = load_w(pool, lw[(li, "WqT")], f"wq{li}")
                wk = load_w(pool, lw[(li, "WkT")], f"wk{li}")
                wv_ = load_w(pool, lw[(li, "WvT")], f"wv{li}")
                wo = load_w(pool, lw[(li, "WoT")], f"wo{li}")
                bq_s = load_b(pool, lw[(li, "bq")], f"bq{li}")
                bk_s = load_b(pool, lw[(li, "bk")], f"bk{li}")
                bo_s = load_b(pool, lw[(li, "bo")], f"bo{li}")

                with tc.For_i(0, TS, hint_engines=(mybir.EngineType.PE,), staggered_reset=True) as iv:
                    xs = pool.tile([128, DC * S], f32r, tag="xs", bufs=1)
                    xv = xs[:].rearrange("p (c f) -> p c f", c=DC)
                    iv8 = xin[:].rearrange("p (c f) -> p c f", c=DC)[:, :, ds(iv * S, S)]
                    for q in range(4):
                        nc.sync.dma_start(out=xv[:, 2 * q:2 * q + 2, :],
                                          in_=iv8[:, 2 * q:2 * q + 2, :])
                    xh = layernorm(pool, pp, xs, S, f"ln{li}a")

                    k_sb = pool.tile([128, DC * S], f32r, tag="k_sb")
                    mm_block(pp, pool, wk, xh, S, bk_s, "bias", k_sb, "k")
                    q_sb = pool.tile([128, DC * NQ], f32r, tag="q_sb")
                    if trim:
                        xhq = pool.tile([128, DC * NQ], f32r, tag="xhq")
                        for c in range(DC):
                            nc.vector.tensor_copy(out=xhq[:, c * NQ:(c + 1) * NQ],
                                                  in_=xh[:, c * S:c * S + NQ])
                        mm_block(pp, pool, wq, xhq, NQ, bq_s, "bias", q_sb, "q")
                    else:
                        mm_block(pp, pool, wq, xh, S, bq_s, "bias", q_sb, "q")

                    # V token-major: rows = sequence positions, cols = 4*256
                    v_sb = [pool.tile([128, D], f32r, tag=f"v_sb{fc}",
                                      name=f"v_sb{fc}") for fc in range(3)]
                    for fc, (f0, fn) in enumerate(((0, 128), (128, 128), (256, 2))):
                        for hp in range(2):
                            ps = pp.tile([128, 512], f32, tag="acc",
                                         name=f"vps{fc}_{hp}", bufs=2)
                            for c in range(DC):
                                nc.tensor.matmul(
                                    ps[0:fn, :],
                                    xh[:, c * S + f0:c * S + f0 + fn],
                                    wv_[c][:, hp * 512:(hp + 1) * 512],
                                    start=(c == 0), stop=(c == DC - 1))
                            nc.vector.tensor_copy(
                                out=v_sb[fc][0:fn, hp * 512:(hp + 1) * 512],
                                in_=ps[0:fn, :])

                    on_sb = pool.tile([128, DC * NQ], f32r, tag="on_sb")
                    kchunks = ((0, 128), (128, 128), (256, 1))
                    for h in range(H):
                        es = pool.tile([128, 2 * NQ], f32r, tag="es",
                                       name=f"es{h}", bufs=2)
                        es2 = pool.tile([1, NQ], f32r, tag="es2",
                                        name=f"es2{h}", bufs=2)
                        for kc, (k0, kn) in enumerate(kchunks):
                            ps_st = pp.tile([128, NQ], f32, tag="acc",
                                            name=f"st{h}_{kc}", bufs=2)
                            for d2 in range(2):
                                kt = 2 * h + d2
                                nc.tensor.matmul(
                                    ps_st[0:kn, :],
                                    k_sb[:, kt * S + k0:kt * S + k0 + kn],
                                    q_sb[:, kt * NQ:(kt + 1) * NQ],
                                    start=(d2 == 0), stop=(d2 == 1))
                            tgt = es[0:kn, kc * NQ:(kc + 1) * NQ] if kc < 2 \
                                else es2[0:1, :]
                            nc.scalar.activation(tgt, ps_st[0:kn, :], Act.Exp)
                        ps_den = pp.tile([1, NQ], f32, tag="den", name=f"den{h}")
                        nc.tensor.matmul(ps_den[:], ones2[:, 0:1],
                                         es[:, 0:NQ], start=True, stop=False)
                        nc.tensor.matmul(ps_den[:], ones2[:, 0:1],
                                         es[:, NQ:2 * NQ], start=False, stop=False)
                        nc.tensor.matmul(ps_den[:], ones2[0:1, 0:1],
                                         es2[0:1, :], start=False, stop=True)
                        recip = pool.tile([1, NQ], f32r, tag="recip",
                                          name=f"recip{h}")
                        with nc.allow_low_precision(reason="softmax denom recip"):
                            nc.vector.reciprocal(recip[:], ps_den[:])
                        ps_rb = pp.tile([128, NQ], f32, tag="rbb", name=f"rb{h}")
                        nc.tensor.matmul(ps_rb[:], ones_r[:], recip[:],
                                         start=True, stop=True)
                        rb_sb = pool.tile([128, NQ], f32, tag="rb_sb",
                                          name=f"rb_sb{h}")
                        nc.scalar.copy(rb_sb[:], ps_rb[:])
                        for d2 in range(2):
                            ps_o = pp.tile([128, NQ], f32, tag="acc",
                                           name=f"o{h}_{d2}", bufs=2)
                            for kc, (k0, kn) in enumerate(kchunks):
                                src = es[0:kn, kc * NQ:(kc + 1) * NQ] if kc < 2 \
                                    else es2[0:1, :]
                                nc.tensor.matmul(
                                    ps_o[:],
                                    v_sb[kc][0:kn, h * 256 + d2 * 128:
                                             h * 256 + d2 * 128 + 128],
                                    src, start=(kc == 0), stop=(kc == 2))
                            cc = 2 * h + d2
                            nc.vector.tensor_tensor(
                                out=on_sb[:, cc * NQ:(cc + 1) * NQ],
                                in0=ps_o[:], in1=rb_sb[:], op=Alu.mult)

                    for cc in range(DC):
                        ps = pp.tile([128, NQ], f32, tag="acc",
                                     name=f"wops{cc}", bufs=2)
                        for dcv in range(DC):
                            nc.tensor.matmul(ps[:],
                                             wo[dcv][:, cc * 128:(cc + 1) * 128],
                                             on_sb[:, dcv * NQ:(dcv + 1) * NQ],
                                             start=(dcv == 0), stop=(dcv == DC - 1))
                        xnew = pool.tile([128, NQ], f32r, tag="xnew",
                                         name=f"xnew{cc}", bufs=3)
                        nc.vector.scalar_tensor_tensor(
                            out=xnew[:], in0=ps[:], scalar=bo_s[:, cc:cc + 1],
                            in1=xs[:, cc * S:cc * S + NQ],
                            op0=Alu.add, op1=Alu.add)
                        if trim:
                            nc.sync.dma_start(out=xFz[:, ds(cc * TS + iv, 1)],
                                              in_=xnew[:, 0:1])
                        else:
                            nc.sync.dma_start(out=xout[:, ds(cc * F + iv * S, S)],
                                              in_=xnew[:])

        def ff_pass(li, xin, xout):
            with tc.tile_pool(name=f"l{li}b", bufs=1) as pool, \
                 tc.tile_pool(name=f"l{li}b_ps", bufs=1, space="PSUM") as pp:
                wf1 = load_w(pool, lw[(li, "Wf1T")], f"wf1{li}")
                wf2 = load_w(pool, lw[(li, "Wf2T")], f"wf2{li}")
                bf1_s = load_b(pool, lw[(li, "bf1")], f"bf1{li}")
                bf2_s = load_b(pool, lw[(li, "bf2")], f"bf2{li}")
                with tc.For_i(0, TS, hint_engines=(mybir.EngineType.PE,), staggered_reset=True) as iv:
                    xs = pool.tile([128, DC * S], f32r, tag="xs", bufs=2)
                    xv = xs[:].rearrange("p (c f) -> p c f", c=DC)
                    iv8 = xin[:].rearrange("p (c f) -> p c f", c=DC)[:, :, ds(iv * S, S)]
                    for q in range(4):
                        nc.sync.dma_start(out=xv[:, 2 * q:2 * q + 2, :],
                                          in_=iv8[:, 2 * q:2 * q + 2, :])
                    xh = layernorm(pool, pp, xs, S, f"ln{li}b")
                    hmid = pool.tile([128, DC * S], f32r, tag="hmid")
                    mm_block(pp, pool, wf1, xh, S, bf1_s, "relu", hmid, "ff1")
                    for cc in range(DC):
                        ps = pp.tile([128, S], f32, tag="acc",
                                     name=f"f2ps{cc}", bufs=2)
                        for dcv in range(DC):
                            nc.tensor.matmul(ps[:],
                                             wf2[dcv][:, cc * 128:(cc + 1) * 128],
                                             hmid[:, dcv * S:(dcv + 1) * S],
                                             start=(dcv == 0), stop=(dcv == DC - 1))
                        xnew = pool.tile([128, S], f32r, tag="xnew",
                                         name=f"fxnew{cc}", bufs=3)
                        nc.vector.scalar_tensor_tensor(
                            out=xnew[:], in0=ps[:], scalar=bf2_s[:, cc:cc + 1],
                            in1=xs[:, cc * S:(cc + 1) * S],
                            op0=Alu.add, op1=Alu.add)
                        nc.sync.dma_start(out=xout[:, ds(cc * F + iv * S, S)],
                                          in_=xnew[:])

        attn_pass(0, xA, xB, trim=False)
        ff_pass(0, xB, xA)
        attn_pass(1, xA, xB, trim=False)
        ff_pass(1, xB, xA)
        attn_pass(2, xA, xB, trim=False)
        ff_pass(2, xB, xA)
        attn_pass(3, xA, None, trim=True)

        # ---------------- final FF + LN + box head on fused cols ---------
        with tc.tile_pool(name="post", bufs=1) as pool, \
             tc.tile_pool(name="post_ps", bufs=1, space="PSUM") as pp:
            xs = pool.tile([128, DC * TS], f32r)
            nc.sync.dma_start(out=xs[:], in_=xFz[:])
            wf1 = load_w(pool, lw[(3, "Wf1T")], "wf13")
            wf2 = load_w(pool, lw[(3, "Wf2T")], "wf23")
            bf1_s = load_b(pool, lw[(3, "bf1")], "bf13")
            bf2_s = load_b(pool, lw[(3, "bf2")], "bf23")
            xh = layernorm(pool, pp, xs, TS, "lnp")
            hmid = pool.tile([128, DC * TS], f32r)
            mm_block(pp, pool, wf1, xh, TS, bf1_s, "relu", hmid, "pf1")
            xfin = pool.tile([128, DC * TS], f32r)
            for cc in range(DC):
                ps = pp.tile([128, TS], f32, tag="acc", name=f"pf2ps{cc}", bufs=2)
                for dcv in range(DC):
                    nc.tensor.matmul(ps[:], wf2[dcv][:, cc * 128:(cc + 1) * 128],
                                     hmid[:, dcv * TS:(dcv + 1) * TS],
                                     start=(dcv == 0), stop=(dcv == DC - 1))
                nc.vector.scalar_tensor_tensor(
                    out=xfin[:, cc * TS:(cc + 1) * TS], in0=ps[:],
                    scalar=bf2_s[:, cc:cc + 1], in1=xs[:, cc * TS:(cc + 1) * TS],
                    op0=Alu.add, op1=Alu.add)
            xhf = layernorm(pool, pp, xfin, TS, "lnp")
            cur = xhf
            for j in (1, 2, 3, 4):
                wj = load_w(pool, box_WT[j], f"bx{j}")
                bj_s = load_b(pool, box_b[j], f"bxb{j}")
                nxt = pool.tile([128, DC * TS], f32r, name=f"bxh{j}")
                mm_block(pp, pool, wj, cur, TS, bj_s, "relu", nxt, f"bx{j}")
                cur = nxt
            w5 = pool.tile([128, DC * 6], f32r)
            nc.sync.dma_start(
                out=w5[:].rearrange("p (c f) -> p c f", c=DC),
                in_=box_W5T[:].rearrange("(c p) f -> p c f", p=128))
            b5_s = pool.tile([6, 1], f32)
            nc.sync.dma_start(out=b5_s[:], in_=box_b5[:])
            ps5 = pp.tile([6, TS], f32, name="ps5")
            for dcv in range(DC):
                nc.tensor.matmul(ps5[:], w5[:, dcv * 6:(dcv + 1) * 6],
                                 cur[:, dcv * TS:(dcv + 1) * TS],
                                 start=(dcv == 0), stop=(dcv == DC - 1))
            ob = pool.tile([6, TS], f32)
            nc.vector.tensor_scalar_add(out=ob[:], in0=ps5[:], scalar1=b5_s[:])
            nc.sync.dma_start(out=out[:], in_=ob[:])

        cpool_cm.__exit__(None, None, None)
        dram_cm.__exit__(None, None, None)

    nc.compile()
    return nc


def _get_compiled():
    global _COMPILED
    if _COMPILED is None:
        _COMPILED = _build()
    return _COMPILED


# ---------------------------------------------------------------------------
# host entry point
# ---------------------------------------------------------------------------

def kernel(grd_token_hidden_states, batch_idx, vision_features_before_mm_projection,
           params):
    from concourse.bass_utils import run_bass_kernel_spmd

    grd = np.asarray(grd_token_hidden_states, dtype=np.float32)
    bidx = np.asarray(batch_idx).astype(np.int64)
    vis = np.asarray(vision_features_before_mm_projection, dtype=np.float32)

    w = _prep_weights(params)
    nc = _get_compiled()

    in_maps = []
    for i in range(N_CORES):
        sl = slice(i * TS, (i + 1) * TS)
        m = dict(w)
        m["grdT"] = _round_f32r(grd[sl].T)
        gathered = vis[bidx[sl]]                      # [TS, L, D_VIS]
        m["visT"] = _round_f32r(
            gathered.transpose(2, 0, 1).reshape(D_VIS, TS * L))
        in_maps.append(m)

    res = run_bass_kernel_spmd(nc, in_maps, core_ids=list(range(N_CORES)))
    outs = [np.asarray(res.results[i]["out"]).T for i in range(N_CORES)]
    return np.concatenate(outs, axis=0).astype(np.float32)


# revision 10
# speedup vs baseline: 1.0638x; 1.0357x over previous
"""Trainium2 Bass kernel for BBoxHeadForGroundTruthBboxRegressionV2.

Strategy
--------
Data-parallel over the 256 ground tokens: 8 NeuronCores x 32 tokens.
Each core runs, fully on-device, for its 32 token-sequences of length
257 (1 lang token + 256 vision latents, padded to 258 columns):

  lang-projection MLP + per-token vision projection  -> x  [1024, 32*258]
  4 pre-LN transformer encoder layers (H=4, dh=256)
  final LN + 5-layer box MLP on the fused (first) token -> [6, 32]

All matmuls run in float32r (fp32 rounded to 11 mantissa bits, processed
at bf16 rate by the PE; exact products, fp32 accumulation).  Activations
are kept feature-major ([d on partitions, tokens on free]); LayerNorm
statistics are computed with ones-matmuls on the PE and broadcast back
across partitions with rank-1 matmuls.  LN gamma/beta, attention scale
and the V bias are folded into weights on the host.  Layer 3 computes
attention only for the fused query column, and the final FF layer + box
head run on the compacted [1024, 32] fused matrix.
"""

import numpy as np

N_CORES = 8
T = 256
D_LM = 4096
D_VIS = 1024
D = 1024
L = 256
B = 16
FF = 1024
H = 4
NL = 4
EPS = 1e-5

TS = T // N_CORES      # tokens per core
S = 258                # padded sequence length (1 lang + 256 vis + 1 pad)
SR = 257               # real sequence length
F = TS * S             # flattened columns per core
DC = D // 128          # feature chunks (8)

_COMPILED = None


def _round_f32r(x):
    """Round float32 array to 11 mantissa bits (RNE) == float32r grid."""
    x = np.ascontiguousarray(x, dtype=np.float32)
    bits = x.view(np.uint32).astype(np.uint64)
    half = np.uint64(2047)
    lsb = (bits >> np.uint64(12)) & np.uint64(1)
    bits = (bits + half + lsb) >> np.uint64(12) << np.uint64(12)
    return bits.astype(np.uint32).view(np.float32)


def _bias_cols(b):
    """[n*128] bias vector -> [128, n] (column per 128-feature chunk)."""
    b = np.asarray(b, dtype=np.float32)
    return np.ascontiguousarray(b.reshape(-1, 128).T)


def _prep_weights(params):
    p = {k: np.asarray(v, dtype=np.float32) for k, v in params.items()}
    w = {}
    r = _round_f32r

    w["vis_WT"] = r(p["vis_W"].T)                      # [D_VIS, D]
    w["vis_b"] = _bias_cols(p["vis_b"])

    w["lang_W1T"] = r(p["lang_W1"].T)                  # [4096, 1024]
    w["lang_W2T"] = r(p["lang_W2"].T)
    w["lang_W3T"] = r(p["lang_W3"].T)
    w["lang_b1"] = _bias_cols(p["lang_b1"])
    w["lang_b2"] = _bias_cols(p["lang_b2"])
    w["lang_b3"] = _bias_cols(p["lang_b3"])

    for i in range(NL):
        g1 = p["enc_ln1_g"][i]
        be1 = p["enc_ln1_b"][i]
        Wqkv = p["enc_Wqkv"][i]
        bqkv = p["enc_bqkv"][i]
        Wq, Wk, Wv = Wqkv[0:D], Wqkv[D:2 * D], Wqkv[2 * D:3 * D]
        bq, bk, bv = bqkv[0:D], bqkv[D:2 * D], bqkv[2 * D:3 * D]

        Wq_f = Wq * g1[None, :]
        b_q = bq + Wq @ be1
        Wk_f = Wk * g1[None, :] / 16.0
        b_k = (bk + Wk @ be1) / 16.0
        Wv_f = Wv * g1[None, :]
        b_v = bv + Wv @ be1
        Wo = p["enc_Wo"][i]
        b_o = p["enc_bo"][i] + Wo @ b_v

        g2 = p["enc_ln2_g"][i]
        be2 = p["enc_ln2_b"][i]
        Wf1_f = p["enc_Wff1"][i] * g2[None, :]
        b_f1 = p["enc_bff1"][i] + p["enc_Wff1"][i] @ be2

        w[f"l{i}_WqT"] = r(Wq_f.T)
        w[f"l{i}_WkT"] = r(Wk_f.T)
        w[f"l{i}_WvT"] = r(Wv_f.T)
        w[f"l{i}_WoT"] = r(Wo.T)
        w[f"l{i}_bq"] = _bias_cols(b_q)
        w[f"l{i}_bk"] = _bias_cols(b_k)
        w[f"l{i}_bo"] = _bias_cols(b_o)
        w[f"l{i}_Wf1T"] = r(Wf1_f.T)
        w[f"l{i}_Wf2T"] = r(p["enc_Wff2"][i].T)
        w[f"l{i}_bf1"] = _bias_cols(b_f1)
        w[f"l{i}_bf2"] = _bias_cols(p["enc_bff2"][i])

    gf = p["enc_lnf_g"]
    bef = p["enc_lnf_b"]
    W1_f = p["box_W1"] * gf[None, :]
    b_1 = p["box_b1"] + p["box_W1"] @ bef
    w["box_W1T"] = r(W1_f.T)
    w["box_b1"] = _bias_cols(b_1)
    for j in (2, 3, 4):
        w[f"box_W{j}T"] = r(p[f"box_W{j}"].T)
        w[f"box_b{j}"] = _bias_cols(p[f"box_b{j}"])
    w["box_W5T"] = r(p["box_W5"].T)                    # [1024, 6]
    w["box_b5"] = np.ascontiguousarray(p["box_b5"].reshape(6, 1))

    w["consts"] = np.stack(
        [np.ones(128, np.float32), np.full(128, 1.0 / 1024.0, np.float32)], axis=1
    )
    w["ones_row"] = np.ones((1, 128), np.float32)
    w["zeros32"] = np.zeros((128, TS), np.float32)
    w["epsb"] = np.full((128, 1), EPS, np.float32)
    return w


# ---------------------------------------------------------------------------
# graph builder
# ---------------------------------------------------------------------------

def _build():
    import concourse.bacc as bacc
    import concourse.mybir as mybir
    from concourse.tile import TileContext
    import concourse.bass as bass

    f32 = mybir.dt.float32
    f32r = mybir.dt.float32r
    Alu = mybir.AluOpType
    Act = mybir.ActivationFunctionType

    nc = bacc.Bacc("TRN2", target_bir_lowering=False, debug=False,
                   num_devices=N_CORES)

    def din(name, shape, dtype=f32r):
        return nc.dram_tensor(name, list(shape), dtype, kind="ExternalInput")

    grdT = din("grdT", [D_LM, TS])
    visT = din("visT", [D_VIS, TS * L])
    consts = din("consts", [128, 2])
    ones_row_d = din("ones_row", [1, 128])
    zeros_d = din("zeros32", [128, TS])
    epsb_d = din("epsb", [128, 1], f32)
    vis_WT = din("vis_WT", [D_VIS, D])
    vis_b = din("vis_b", [128, DC], f32)
    lang_W1T = din("lang_W1T", [D_LM, FF])
    lang_W2T = din("lang_W2T", [FF, FF])
    lang_W3T = din("lang_W3T", [FF, D])
    lang_b = [din(f"lang_b{j}", [128, DC], f32) for j in (1, 2, 3)]
    lw = {}
    for i in range(NL):
        for nm in ("WqT", "WkT", "WvT", "WoT", "Wf1T", "Wf2T"):
            lw[(i, nm)] = din(f"l{i}_{nm}", [D, D])
        for nm in ("bq", "bk", "bo", "bf1", "bf2"):
            lw[(i, nm)] = din(f"l{i}_{nm}", [128, DC], f32)
    box_WT = {j: din(f"box_W{j}T", [D, D]) for j in (1, 2, 3, 4)}
    box_b = {j: din(f"box_b{j}", [128, DC], f32) for j in (1, 2, 3, 4)}
    box_W5T = din("box_W5T", [D, 6])
    box_b5 = din("box_b5", [6, 1], f32)

    out = nc.dram_tensor("out", [6, TS], f32, kind="ExternalOutput")

    ds = bass.ds

    with TileContext(nc) as tc:
        dram_cm = tc.tile_pool(name="dram", bufs=1, space="DRAM")
        dram = dram_cm.__enter__()
        xA = dram.tile([128, DC * F], f32r, name="xA")
        xB = dram.tile([128, DC * F], f32r, name="xB")
        xFz = dram.tile([128, DC * TS], f32r, name="xFz")   # fused cols after L3A

        cpool_cm = tc.tile_pool(name="cpool", bufs=1)
        cpool = cpool_cm.__enter__()
        ones2 = cpool.tile([128, 2], f32r)
        nc.sync.dma_start(out=ones2[:], in_=consts[:])
        ones_r = cpool.tile([1, 128], f32r)
        nc.sync.dma_start(out=ones_r[:], in_=ones_row_d[:])
        epsb = cpool.tile([128, 1], f32)
        nc.sync.dma_start(out=epsb[:], in_=epsb_d[:])

        # ---------------- helpers ----------------------------------------
        def layernorm(pool, pp, xs, n, tag):
            """feature-major LN over DC chunks of xs ([128, DC*n]); returns
            standardized xh [128, DC*n] f32r (gamma/beta folded into the
            following matmul's weights on the host)."""
            ps_mu = pp.tile([1, n], f32, tag=f"{tag}_mu", name=f"{tag}_mu")
            ps_s2 = pp.tile([1, n], f32, tag=f"{tag}_s2", name=f"{tag}_s2")
            for c in range(DC):
                nc.tensor.matmul(ps_mu[:], ones2[:, 1:2], xs[:, c * n:(c + 1) * n],
                                 start=(c == 0), stop=(c == DC - 1))
            for c in range(DC):
                xsq = pool.tile([128, n], f32r, tag=f"{tag}_xsq",
                                name=f"{tag}_xsq{c}", bufs=2)
                nc.scalar.square(xsq[:], xs[:, c * n:(c + 1) * n])
                nc.tensor.matmul(ps_s2[:], ones2[:, 1:2], xsq[:],
                                 start=(c == 0), stop=(c == DC - 1))
            mu = pool.tile([1, n], f32r, tag=f"{tag}_musb", name=f"{tag}_musb")
            nc.scalar.copy(mu[:], ps_mu[:])
            musq = pool.tile([1, n], f32, tag=f"{tag}_musq", name=f"{tag}_musq")
            nc.scalar.square(musq[:], mu[:])
            var = pool.tile([1, n], f32, tag=f"{tag}_var", name=f"{tag}_var")
            nc.vector.tensor_tensor(out=var[:], in0=ps_s2[:], in1=musq[:],
                                    op=Alu.subtract)
            sd = pool.tile([1, n], f32, tag=f"{tag}_sd", name=f"{tag}_sd")
            nc.scalar.activation(sd[:], var[:], Act.Sqrt, bias=epsb[0:1, :])
            rstd = pool.tile([1, n], f32r, tag=f"{tag}_rstd", name=f"{tag}_rstd")
            with nc.allow_low_precision(reason="f32r rstd for broadcast matmul"):
                nc.vector.reciprocal(rstd[:], sd[:])
            ps_mub = pp.tile([128, n], f32, tag="acc", name=f"{tag}_mub", bufs=6)
            nc.tensor.matmul(ps_mub[:], ones_r[:], mu[:], start=True, stop=True)
            ps_rb = pp.tile([128, n], f32, tag="acc", name=f"{tag}_rb", bufs=6)
            nc.tensor.matmul(ps_rb[:], ones_r[:], rstd[:], start=True, stop=True)
            xh = pool.tile([128, DC * n], f32r, tag=f"{tag}_xh", name=f"{tag}_xh")
            for c in range(DC):
                td = pool.tile([128, n], f32, tag=f"{tag}_td",
                               name=f"{tag}_td{c}", bufs=2)
                nc.vector.tensor_tensor(out=td[:], in0=xs[:, c * n:(c + 1) * n],
                                        in1=ps_mub[:], op=Alu.subtract)
                nc.vector.tensor_tensor(out=xh[:, c * n:(c + 1) * n], in0=td[:],
                                        in1=ps_rb[:], op=Alu.mult)
            return xh

        def mm_block(pp, pool, wtiles, rhs_tile, n, bias, act, out_tile, tag,
                     n_dc=DC, n_cc=DC):
            """out[:, cc*n:(cc+1)*n] =
            act(sum_dc wtiles[dc][:, cc*128:+128].T @ rhs[:, dc*n:+n] + bias[cc])"""
            for cc in range(n_cc):
                ps = pp.tile([128, n], f32, tag="acc", name=f"{tag}_ps{cc}", bufs=6)
                for dc in range(n_dc):
                    nc.tensor.matmul(ps[:], wtiles[dc][:, cc * 128:(cc + 1) * 128],
                                     rhs_tile[:, dc * n:(dc + 1) * n],
                                     start=(dc == 0), stop=(dc == n_dc - 1))
                dst = out_tile[:, cc * n:(cc + 1) * n]
                if act == "relu":
                    nc.scalar.activation(dst, ps[:], Act.Relu, bias=bias[:, cc:cc + 1])
                elif act == "bias":
                    nc.vector.tensor_scalar_add(out=dst, in0=ps[:],
                                                scalar1=bias[:, cc:cc + 1])
                else:
                    nc.vector.tensor_copy(out=dst, in_=ps[:])

        class _WView:
            def __init__(self, tile, ncols):
                self.tile = tile
                self.ncols = ncols
            def __getitem__(self, dcv):
                return self.tile[:, dcv * self.ncols:(dcv + 1) * self.ncols]

        def load_w(pool, dram_t, name, ncols=D):
            t = pool.tile([128, DC * ncols], f32r, tag=name, name=name)
            tv = t[:].rearrange("p (c f) -> p c f", c=DC)
            dv = dram_t[:].rearrange("(c p) f -> p c f", p=128)
            for q in range(4):
                nc.sync.dma_start(out=tv[:, 2 * q:2 * q + 2, :],
                                  in_=dv[:, 2 * q:2 * q + 2, :])
            return _WView(t, ncols)

        def load_b(pool, dram_t, name):
            t = pool.tile([128, DC], f32, name=name)
            nc.sync.dma_start(out=t[:], in_=dram_t[:])
            return t

        # ---------------- stage 0: lang MLP ------------------------------
        with tc.tile_pool(name="lang", bufs=1) as pool, \
             tc.tile_pool(name="lang_ps", bufs=1, space="PSUM") as pp:
            g_sb = pool.tile([128, 32 * TS], f32r)
            nc.sync.dma_start(
                out=g_sb[:].rearrange("p (c f) -> p c f", c=32),
                in_=grdT[:].rearrange("(c p) f -> p c f", p=128))
            w1t = pool.tile([128, 32 * FF], f32r, name="lw1")
            w1v = w1t[:].rearrange("p (c f) -> p c f", c=32)
            l1v = lang_W1T[:].rearrange("(c p) f -> p c f", p=128)
            for q in range(8):
                nc.sync.dma_start(out=w1v[:, 4 * q:4 * q + 4, :],
                                  in_=l1v[:, 4 * q:4 * q + 4, :])
            w1 = _WView(w1t, FF)
            b1 = load_b(pool, lang_b[0], "lb1")
            b2 = load_b(pool, lang_b[1], "lb2")
            b3 = load_b(pool, lang_b[2], "lb3")
            h1 = pool.tile([128, DC * TS], f32r)
            mm_block(pp, pool, w1, g_sb, TS, b1, "relu", h1, "lg1", n_dc=32)
            w2 = load_w(pool, lang_W2T, "lw2", FF)
            h2 = pool.tile([128, DC * TS], f32r)
            mm_block(pp, pool, w2, h1, TS, b2, "relu", h2, "lg2")
            w3 = load_w(pool, lang_W3T, "lw3", D)
            lang_o = pool.tile([128, DC * TS], f32r)
            mm_block(pp, pool, w3, h2, TS, b3, "bias", lang_o, "lg3")
            zt = pool.tile([128, TS], f32r)
            nc.sync.dma_start(out=zt[:], in_=zeros_d[:])
            for c in range(DC):
                xa_c = xA[:, c * F:(c + 1) * F].rearrange("p (t s) -> p t s", s=S)
                nc.sync.dma_start(out=xa_c[:, :, 0:1],
                                  in_=lang_o[:, c * TS:(c + 1) * TS])
                nc.sync.dma_start(out=xa_c[:, :, SR:S], in_=zt[:])

        # ---------------- stage 1: vision projection ---------------------
        with tc.tile_pool(name="vis", bufs=1) as pool, \
             tc.tile_pool(name="vis_ps", bufs=1, space="PSUM") as pp:
            wv = load_w(pool, vis_WT, "visw", D)
            vb = load_b(pool, vis_b, "visb")
            with tc.For_i(0, TS, hint_engines=(mybir.EngineType.PE,), staggered_reset=True) as iv:
                v_in = pool.tile([128, DC * L], f32r, tag="v_in", bufs=2)
                for c in range(DC):
                    nc.sync.dma_start(out=v_in[:, c * L:(c + 1) * L],
                                      in_=visT[c * 128:(c + 1) * 128, ds(iv * L, L)])
                v_out = pool.tile([128, DC * L], f32r, tag="v_out", bufs=2)
                mm_block(pp, pool, wv, v_in, L, vb, "bias", v_out, "vis")
                for c in range(DC):
                    nc.sync.dma_start(out=xA[:, ds(c * F + iv * S + 1, L)],
                                      in_=v_out[:, c * L:(c + 1) * L])

        # ---------------- encoder layers ---------------------------------
        def attn_pass(li, xin, xout, trim):
            NQ = 2 if trim else S
            with tc.tile_pool(name=f"l{li}a", bufs=1) as pool, \
                 tc.tile_pool(name=f"l{li}a_ps", bufs=1, space="PSUM") as pp:
                wq = load_w(pool, lw[(li, "WqT")], f"wq{li}")
                wk = load_w(pool, lw[(li, "WkT")], f"wk{li}")
                wv_ = load_w(pool, lw[(li, "WvT")], f"wv{li}")
                wo = load_w(pool, lw[(li, "WoT")], f"wo{li}")
                bq_s = load_b(pool, lw[(li, "bq")], f"bq{li}")
                bk_s = load_b(pool, lw[(li, "bk")], f"bk{li}")
                bo_s = load_b(pool, lw[(li, "bo")], f"bo{li}")

                with tc.For_i(0, TS, hint_engines=(mybir.EngineType.PE,), staggered_reset=True) as iv:
                    xs = pool.tile([128, DC * S], f32r, tag="xs", bufs=1)
                    xv = xs[:].rearrange("p (c f) -> p c f", c=DC)
                    iv8 = xin[:].rearrange("p (c f) -> p c f", c=DC)[:, :, ds(iv * S, S)]
                    for q in range(4):
                        nc.sync.dma_start(out=xv[:, 2 * q:2 * q + 2, :],
                                          in_=iv8[:, 2 * q:2 * q + 2, :])
                    xh = layernorm(pool, pp, xs, S, f"ln{li}a")

                    k_sb = pool.tile([128, DC * S], f32r, tag="k_sb")
                    mm_block(pp, pool, wk, xh, S, bk_s, "bias", k_sb, "k")
                    q_sb = pool.tile([128, DC * NQ], f32r, tag="q_sb")
                    if trim:
                        xhq = pool.tile([128, DC * NQ], f32r, tag="xhq")
                        for c in range(DC):
                            nc.vector.tensor_copy(out=xhq[:, c * NQ:(c + 1) * NQ],
                                                  in_=xh[:, c * S:c * S + NQ])
                        mm_block(pp, pool, wq, xhq, NQ, bq_s, "bias", q_sb, "q")
                    else:
                        mm_block(pp, pool, wq, xh, S, bq_s, "bias", q_sb, "q")

                    # V token-major: rows = sequence positions, cols = 4*256
                    v_sb = [pool.tile([128, D], f32r, tag=f"v_sb{fc}",
                                      name=f"v_sb{fc}") for fc in range(3)]
                    for fc, (f0, fn) in enumerate(((0, 128), (128, 128), (256, 2))):
                        for hp in range(2):
                            ps = pp.tile([128, 512], f32, tag="acc",
                                         name=f"vps{fc}_{hp}", bufs=6)
                            for c in range(DC):
                                nc.tensor.matmul(
                                    ps[0:fn, :],
                                    xh[:, c * S + f0:c * S + f0 + fn],
                                    wv_[c][:, hp * 512:(hp + 1) * 512],
                                    start=(c == 0), stop=(c == DC - 1))
                            nc.vector.tensor_copy(
                                out=v_sb[fc][0:fn, hp * 512:(hp + 1) * 512],
                                in_=ps[0:fn, :])

                    on_sb = pool.tile([128, DC * NQ], f32r, tag="on_sb")
                    kchunks = ((0, 128), (128, 128), (256, 1))
                    for h in range(H):
                        es = pool.tile([128, 2 * NQ], f32r, tag="es",
                                       name=f"es{h}", bufs=2)
                        es2 = pool.tile([1, NQ], f32r, tag="es2",
                                        name=f"es2{h}", bufs=2)
                        for kc, (k0, kn) in enumerate(kchunks):
                            ps_st = pp.tile([128, NQ], f32, tag="acc",
                                            name=f"st{h}_{kc}", bufs=6)
                            for d2 in range(2):
                                kt = 2 * h + d2
                                nc.tensor.matmul(
                                    ps_st[0:kn, :],
                                    k_sb[:, kt * S + k0:kt * S + k0 + kn],
                                    q_sb[:, kt * NQ:(kt + 1) * NQ],
                                    start=(d2 == 0), stop=(d2 == 1))
                            tgt = es[0:kn, kc * NQ:(kc + 1) * NQ] if kc < 2 \
                                else es2[0:1, :]
                            nc.scalar.activation(tgt, ps_st[0:kn, :], Act.Exp)
                        ps_den = pp.tile([1, NQ], f32, tag="acc", name=f"den{h}", bufs=6)
                        nc.tensor.matmul(ps_den[:], ones2[:, 0:1],
                                         es[:, 0:NQ], start=True, stop=False)
                        nc.tensor.matmul(ps_den[:], ones2[:, 0:1],
                                         es[:, NQ:2 * NQ], start=False, stop=False)
                        nc.tensor.matmul(ps_den[:], ones2[0:1, 0:1],
                                         es2[0:1, :], start=False, stop=True)
                        recip = pool.tile([1, NQ], f32r, tag="recip",
                                          name=f"recip{h}")
                        with nc.allow_low_precision(reason="softmax denom recip"):
                            nc.vector.reciprocal(recip[:], ps_den[:])
                        ps_rb = pp.tile([128, NQ], f32, tag="acc", name=f"rb{h}", bufs=6)
                        nc.tensor.matmul(ps_rb[:], ones_r[:], recip[:],
                                         start=True, stop=True)
                        rb_sb = pool.tile([128, NQ], f32, tag="rb_sb",
                                          name=f"rb_sb{h}")
                        nc.scalar.copy(rb_sb[:], ps_rb[:])
                        for d2 in range(2):
                            ps_o = pp.tile([128, NQ], f32, tag="acc",
                                           name=f"o{h}_{d2}", bufs=6)
                            for kc, (k0, kn) in enumerate(kchunks):
                                src = es[0:kn, kc * NQ:(kc + 1) * NQ] if kc < 2 \
                                    else es2[0:1, :]
                                nc.tensor.matmul(
                                    ps_o[:],
                                    v_sb[kc][0:kn, h * 256 + d2 * 128:
                                             h * 256 + d2 * 128 + 128],
                                    src, start=(kc == 0), stop=(kc == 2))
                            cc = 2 * h + d2
                            nc.vector.tensor_tensor(
                                out=on_sb[:, cc * NQ:(cc + 1) * NQ],
                                in0=ps_o[:], in1=rb_sb[:], op=Alu.mult)

                    for cc in range(DC):
                        ps = pp.tile([128, NQ], f32, tag="acc",
                                     name=f"wops{cc}", bufs=6)
                        for dcv in range(DC):
                            nc.tensor.matmul(ps[:],
                                             wo[dcv][:, cc * 128:(cc + 1) * 128],
                                             on_sb[:, dcv * NQ:(dcv + 1) * NQ],
                                             start=(dcv == 0), stop=(dcv == DC - 1))
                        xnew = pool.tile([128, NQ], f32r, tag="xnew",
                                         name=f"xnew{cc}", bufs=3)
                        nc.vector.scalar_tensor_tensor(
                            out=xnew[:], in0=ps[:], scalar=bo_s[:, cc:cc + 1],
                            in1=xs[:, cc * S:cc * S + NQ],
                            op0=Alu.add, op1=Alu.add)
                        if trim:
                            nc.sync.dma_start(out=xFz[:, ds(cc * TS + iv, 1)],
                                              in_=xnew[:, 0:1])
                        else:
                            nc.sync.dma_start(out=xout[:, ds(cc * F + iv * S, S)],
                                              in_=xnew[:])

        def ff_pass(li, xin, xout):
            with tc.tile_pool(name=f"l{li}b", bufs=1) as pool, \
                 tc.tile_pool(name=f"l{li}b_ps", bufs=1, space="PSUM") as pp:
                wf1 = load_w(pool, lw[(li, "Wf1T")], f"wf1{li}")
                wf2 = load_w(pool, lw[(li, "Wf2T")], f"wf2{li}")
                bf1_s = load_b(pool, lw[(li, "bf1")], f"bf1{li}")
                bf2_s = load_b(pool, lw[(li, "bf2")], f"bf2{li}")
                with tc.For_i(0, TS, hint_engines=(mybir.EngineType.PE,), staggered_reset=True) as iv:
                    xs = pool.tile([128, DC * S], f32r, tag="xs", bufs=2)
                    xv = xs[:].rearrange("p (c f) -> p c f", c=DC)
                    iv8 = xin[:].rearrange("p (c f) -> p c f", c=DC)[:, :, ds(iv * S, S)]
                    for q in range(4):
                        nc.sync.dma_start(out=xv[:, 2 * q:2 * q + 2, :],
                                          in_=iv8[:, 2 * q:2 * q + 2, :])
                    xh = layernorm(pool, pp, xs, S, f"ln{li}b")
                    hmid = pool.tile([128, DC * S], f32r, tag="hmid")
                    mm_block(pp, pool, wf1, xh, S, bf1_s, "relu", hmid, "ff1")
                    for cc in range(DC):
                        ps = pp.tile([128, S], f32, tag="acc",
                                     name=f"f2ps{cc}", bufs=6)
                        for dcv in range(DC):
                            nc.tensor.matmul(ps[:],
                                             wf2[dcv][:, cc * 128:(cc + 1) * 128],
                                             hmid[:, dcv * S:(dcv + 1) * S],
                                             start=(dcv == 0), stop=(dcv == DC - 1))
                        xnew = pool.tile([128, S], f32r, tag="xnew",
                                         name=f"fxnew{cc}", bufs=3)
                        nc.vector.scalar_tensor_tensor(
                            out=xnew[:], in0=ps[:], scalar=bf2_s[:, cc:cc + 1],
                            in1=xs[:, cc * S:(cc + 1) * S],
                            op0=Alu.add, op1=Alu.add)
                        nc.sync.dma_start(out=xout[:, ds(cc * F + iv * S, S)],
                                          in_=xnew[:])

        attn_pass(0, xA, xB, trim=False)
        ff_pass(0, xB, xA)
        attn_pass(1, xA, xB, trim=False)
        ff_pass(1, xB, xA)
        attn_pass(2, xA, xB, trim=False)
        ff_pass(2, xB, xA)
        attn_pass(3, xA, None, trim=True)

        # ---------------- final FF + LN + box head on fused cols ---------
        with tc.tile_pool(name="post", bufs=1) as pool, \
             tc.tile_pool(name="post_ps", bufs=1, space="PSUM") as pp:
            xs = pool.tile([128, DC * TS], f32r)
            nc.sync.dma_start(out=xs[:], in_=xFz[:])
            wf1 = load_w(pool, lw[(3, "Wf1T")], "wf13")
            wf2 = load_w(pool, lw[(3, "Wf2T")], "wf23")
            bf1_s = load_b(pool, lw[(3, "bf1")], "bf13")
            bf2_s = load_b(pool, lw[(3, "bf2")], "bf23")
            xh = layernorm(pool, pp, xs, TS, "lnp")
            hmid = pool.tile([128, DC * TS], f32r)
            mm_block(pp, pool, wf1, xh, TS, bf1_s, "relu", hmid, "pf1")
            xfin = pool.tile([128, DC * TS], f32r)
            for cc in range(DC):
                ps = pp.tile([128, TS], f32, tag="acc", name=f"pf2ps{cc}", bufs=6)
                for dcv in range(DC):
                    nc.tensor.matmul(ps[:], wf2[dcv][:, cc * 128:(cc + 1) * 128],
                                     hmid[:, dcv * TS:(dcv + 1) * TS],
                                     start=(dcv == 0), stop=(dcv == DC - 1))
                nc.vector.scalar_tensor_tensor(
                    out=xfin[:, cc * TS:(cc + 1) * TS], in0=ps[:],
                    scalar=bf2_s[:, cc:cc + 1], in1=xs[:, cc * TS:(cc + 1) * TS],
                    op0=Alu.add, op1=Alu.add)
            xhf = layernorm(pool, pp, xfin, TS, "lnp")
            cur = xhf
            for j in (1, 2, 3, 4):
                wj = load_w(pool, box_WT[j], f"bx{j}")
                bj_s = load_b(pool, box_b[j], f"bxb{j}")
                nxt = pool.tile([128, DC * TS], f32r, name=f"bxh{j}")
                mm_block(pp, pool, wj, cur, TS, bj_s, "relu", nxt, f"bx{j}")
                cur = nxt
            w5 = pool.tile([128, DC * 6], f32r)
            nc.sync.dma_start(
                out=w5[:].rearrange("p (c f) -> p c f", c=DC),
                in_=box_W5T[:].rearrange("(c p) f -> p c f", p=128))
            b5_s = pool.tile([6, 1], f32)
            nc.sync.dma_start(out=b5_s[:], in_=box_b5[:])
            ps5 = pp.tile([6, TS], f32, tag="acc", name="ps5", bufs=6)
            for dcv in range(DC):
                nc.tensor.matmul(ps5[:], w5[:, dcv * 6:(dcv + 1) * 6],
                                 cur[:, dcv * TS:(dcv + 1) * TS],
                                 start=(dcv == 0), stop=(dcv == DC - 1))
            ob = pool.tile([6, TS], f32)
            nc.vector.tensor_scalar_add(out=ob[:], in0=ps5[:], scalar1=b5_s[:])
            nc.sync.dma_start(out=out[:], in_=ob[:])

        cpool_cm.__exit__(None, None, None)
        dram_cm.__exit__(None, None, None)

    nc.compile()
    return nc


def _get_compiled():
    global _COMPILED
    if _COMPILED is None:
        _COMPILED = _build()
    return _COMPILED


# ---------------------------------------------------------------------------
# host entry point
# ---------------------------------------------------------------------------

def kernel(grd_token_hidden_states, batch_idx, vision_features_before_mm_projection,
           params):
    from concourse.bass_utils import run_bass_kernel_spmd

    grd = np.asarray(grd_token_hidden_states, dtype=np.float32)
    bidx = np.asarray(batch_idx).astype(np.int64)
    vis = np.asarray(vision_features_before_mm_projection, dtype=np.float32)

    w = _prep_weights(params)
    nc = _get_compiled()

    in_maps = []
    for i in range(N_CORES):
        sl = slice(i * TS, (i + 1) * TS)
        m = dict(w)
        m["grdT"] = _round_f32r(grd[sl].T)
        gathered = vis[bidx[sl]]                      # [TS, L, D_VIS]
        m["visT"] = _round_f32r(
            gathered.transpose(2, 0, 1).reshape(D_VIS, TS * L))
        in_maps.append(m)

    res = run_bass_kernel_spmd(nc, in_maps, core_ids=list(range(N_CORES)))
    outs = [np.asarray(res.results[i]["out"]).T for i in range(N_CORES)]
    return np.concatenate(outs, axis=0).astype(np.float32)
